# revision 3
# baseline (speedup 1.0000x reference)
"""MoELayer Trainium2 kernel (8 NeuronCores, SPMD).

Strategy:
  - Router matmul row-sharded over in_dim: each core computes partial scores
    for ALL 64 samples over its 25088-wide slice (fp32, exact), then a
    ReduceScatter(add) hands each core the final scores of its own 8 samples.
  - Exact top-128 per sample via bit-bisection on |scores| (int32 view of
    fp32 is order-isomorphic for non-negative floats), with jax.top_k tie
    semantics (lowest index wins) via an equality-cumsum pass.
  - Per-sample one-hot selection matrix S [512, 128] built on DVE; conv
    weights gathered as w_sel = wa.T @ S with float32r matmuls (values are
    0/1 so S is exact; weights round to f32r ~13-bit mantissa).
  - 3x3 conv on the 128 selected channels only (4x compute saving) in
    float32r: "double image" SBUF layout xx = [x_pad ; x_pad shifted one row]
    so (dy=0, dy=1) pack into one K=128 matmul; dy=2 runs as K=64 matmuls
    alternating between the two partition halves (row-tiling overlap).
  - PSUM drained by ScalarE with fused per-channel bias add.

Batch is data-parallel: core r owns samples [8r, 8r+8).
"""
import numpy as np

import concourse.bacc as bacc
import concourse.bass as bass
import concourse.mybir as mybir
import concourse.tile as tile
from concourse.bass_utils import run_bass_kernel_spmd

F32 = mybir.dt.float32
F32R = mybir.dt.float32r
I32 = mybir.dt.int32
OP = mybir.AluOpType
AFT = mybir.ActivationFunctionType

B, CIN, H, W = 64, 64, 56, 56
COUT, NEXP = 128, 4
CH = NEXP * COUT            # 512
IN_DIM = CIN * H * W        # 200704
NCORES = 8
BS = B // NCORES            # 8 samples per core
KC = IN_DIM // NCORES // 128  # 196 k-chunks of 128 per core
HP = H + 2                  # 58 padded
NPAD = HP * HP              # 3364
RT = 8                      # row-tiles per sample (7 output rows each)
RPT = H // RT               # 7 rows per tile
NT = RPT * W                # 392 columns per conv matmul


def build_nc():
    nc = bacc.Bacc("TRN2", target_bir_lowering=False, debug=False,
                   num_devices=NCORES)

    rw = nc.dram_tensor("rw", [KC, 128, CH], F32, kind="ExternalInput")
    xr = nc.dram_tensor("xr", [128, KC, B], F32, kind="ExternalInput")
    xc = nc.dram_tensor("xc", [BS, CIN, H, W], F32, kind="ExternalInput")
    wa = nc.dram_tensor("wa", [4, 128, 768], F32, kind="ExternalInput")
    cb = nc.dram_tensor("cb", [4, 128, 1], F32, kind="ExternalInput")
    rb = nc.dram_tensor("rb", [BS, CH], F32, kind="ExternalInput")
    eye8 = nc.dram_tensor("eye8", [8, 8], F32, kind="ExternalInput")
    iotaj = nc.dram_tensor("iotaj", [128, 128], F32, kind="ExternalInput")
    out = nc.dram_tensor("out", [BS, COUT, H, W], F32, kind="ExternalOutput")

    with tile.TileContext(nc) as tc:
        with (
            tc.tile_pool(name="sb", bufs=1) as sb,
            tc.tile_pool(name="sbrw", bufs=4) as sbrw,
            tc.tile_pool(name="sbxx", bufs=2) as sbxx,
            tc.tile_pool(name="sbot", bufs=3) as sbot,
            tc.tile_pool(name="dram", bufs=1, space="DRAM") as dram,
            tc.tile_pool(name="ps_sc", bufs=1, space="PSUM") as ps_sc,
            tc.tile_pool(name="ps_tr", bufs=1, space="PSUM") as ps_tr,
            tc.tile_pool(name="ps_ws", bufs=2, space="PSUM") as ps_ws,
            tc.tile_pool(name="ps_cv", bufs=3, space="PSUM") as ps_cv,
        ):
            # ---------------- constants / static loads ----------------
            eyet = sb.tile([8, 8], F32, tag="eye")
            nc.sync.dma_start(eyet[:], eye8.ap())
            iott = sb.tile([128, 128], F32, tag="iot")
            nc.sync.dma_start(iott[:], iotaj.ap())
            rbt = sb.tile([BS, CH], F32, tag="rb")
            nc.sync.dma_start(rbt[:], rb.ap())
            wat = sb.tile([128, 4, 768], F32R, tag="wa")
            for c in range(4):
                nc.gpsimd.dma_start(wat[:, c, :], wa.ap()[c])
            cbt = sb.tile([128, 4, 1], F32, tag="cb")
            for c in range(4):
                nc.sync.dma_start(cbt[:, c, :], cb.ap()[c])

            # ---------------- phase R: router partial scores ----------------
            xrt = sb.tile([128, KC, B], F32, tag="xr")
            nc.sync.dma_start(xrt[:], xr.ap())

            psc = ps_sc.tile([B, CH], F32, tag="psc")
            for k in range(KC):
                rwk = sbrw.tile([128, CH], F32, tag="rwk")
                nc.sync.dma_start(rwk[:], rw.ap()[k])
                nc.tensor.matmul(psc[:], xrt[:, k, :], rwk[:],
                                 start=(k == 0), stop=(k == KC - 1))
            scp = sb.tile([B, CH], F32, tag="scp")
            nc.vector.tensor_copy(scp[:], psc[:])

            rs_in = dram.tile([B, CH], F32)
            rs_out = dram.tile([BS, CH], F32)
            nc.sync.dma_start(rs_in[:], scp[:])
            nc.gpsimd.collective_compute(
                "ReduceScatter", OP.add,
                replica_groups=[list(range(NCORES))],
                ins=[rs_in.opt()], outs=[rs_out.opt()],
            )
            scf = sb.tile([BS, CH], F32, tag="scf")
            nc.sync.dma_start(scf[:], rs_out[:])
            nc.vector.tensor_tensor(scf[:], scf[:], rbt[:], OP.add)

            # ---------------- phase T: exact top-128 ----------------
            sa = sb.tile([BS, CH], F32, tag="sa")
            nc.scalar.activation(sa[:], scf[:], AFT.Abs)
            lo = sb.tile([BS, 1], I32, tag="lo")
            nc.vector.memset(lo[:], 0)
            cand = sb.tile([BS, 1], I32, tag="cand")
            msk = sb.tile([BS, CH], F32, tag="msk")
            cnt = sb.tile([BS, 1], F32, tag="cnt")
            flag = sb.tile([BS, 1], F32, tag="flag")
            stpi = sb.tile([BS, 1], I32, tag="stpi")
            for b in range(30, -1, -1):
                nc.vector.tensor_scalar(cand[:], lo[:], (1 << b), None, OP.add)
                nc.vector.tensor_scalar(msk[:], sa[:], cand[:].bitcast(F32),
                                        None, OP.is_ge, OP.add,
                                        accum_out=cnt[:])
                nc.vector.tensor_scalar(flag[:], cnt[:], float(COUT), None,
                                        OP.is_ge)
                nc.vector.tensor_scalar(flag[:], flag[:], float(1 << b), None,
                                        OP.mult)
                nc.vector.tensor_copy(stpi[:], flag[:])
                nc.vector.tensor_tensor(lo[:], lo[:], stpi[:], OP.add)
            # lo == bits of the 128th largest |score| per sample
            mgt = sb.tile([BS, CH], F32, tag="mgt")
            ngt = sb.tile([BS, 1], F32, tag="ngt")
            nc.vector.tensor_scalar(mgt[:], sa[:], lo[:].bitcast(F32), None,
                                    OP.is_gt, OP.add, accum_out=ngt[:])
            meq = sb.tile([BS, CH], F32, tag="meq")
            nc.vector.tensor_scalar(meq[:], sa[:], lo[:].bitcast(F32), None,
                                    OP.is_equal)
            need = sb.tile([BS, 1], F32, tag="need")
            nc.vector.tensor_scalar(need[:], ngt[:], -1.0, None, OP.mult)
            nc.vector.tensor_scalar(need[:], need[:], float(COUT), None, OP.add)
            zf = sb.tile([BS, CH], F32, tag="zf")
            nc.vector.memset(zf[:], 0.0)
            cume = sb.tile([BS, CH], F32, tag="cume")
            nc.vector.tensor_tensor_scan(cume[:], meq[:], zf[:], 0.0,
                                         OP.add, OP.add)
            keep = sb.tile([BS, CH], F32, tag="keep")
            nc.vector.tensor_scalar(keep[:], cume[:], need[:], None, OP.is_le)
            nc.vector.tensor_tensor(keep[:], keep[:], meq[:], OP.mult)
            nc.vector.tensor_tensor(msk[:], mgt[:], keep[:], OP.add)
            cum = sb.tile([BS, CH], F32, tag="cum")
            nc.vector.tensor_tensor_scan(cum[:], msk[:], zf[:], 0.0,
                                         OP.add, OP.add)
            pos = sb.tile([BS, CH], F32, tag="pos")
            nc.vector.tensor_tensor(pos[:], cum[:], msk[:], OP.mult)
            nc.vector.tensor_scalar(pos[:], pos[:], -1.0, None, OP.add)

            posT = sb.tile([128, 4, BS], F32, tag="posT")
            for c in range(4):
                ptr = ps_tr.tile([128, BS], F32, tag="ptr")
                nc.tensor.transpose(ptr[:], pos[:, c * 128:(c + 1) * 128],
                                    eyet[:])
                nc.vector.tensor_copy(posT[:, c, :], ptr[:])

            # ---------------- phase S: selection matrices + weight gather ----
            S = sb.tile([128, 4, BS, 128], F32R, tag="S")
            S32 = sb.tile([128, 4, BS, 128], F32, tag="S32")
            for c in range(4):
                for s in range(BS):
                    nc.vector.tensor_scalar(S[:, c, s, :], iott[:],
                                            posT[:, c, s:s + 1], None,
                                            OP.is_equal)
                    nc.vector.tensor_scalar(S32[:, c, s, :], iott[:],
                                            posT[:, c, s:s + 1], None,
                                            OP.is_equal)
            # per-sample selected bias [128j, 1]
            bsel = sb.tile([128, BS], F32, tag="bsel")
            for s in range(BS):
                pb = ps_tr.tile([128, 1], F32, tag="pb")
                for c in range(4):
                    nc.tensor.matmul(pb[:], S32[:, c, s, :], cbt[:, c, :],
                                     start=(c == 0), stop=(c == 3))
                nc.scalar.copy(bsel[:, s:s + 1], pb[:])
            # gathered weights wsel[m-chunk][s][j], m-chunk partition = stacks
            wsel = sb.tile([128, 6, BS, 128], F32R, tag="wsel")
            for g in range(4):          # groups of 2 samples (N=256)
                for m in range(6):
                    pw = ps_ws.tile([128, 2, 128], F32, tag="pw")
                    for c in range(4):
                        nc.tensor.matmul(
                            pw[:], wat[:, c, m * 128:(m + 1) * 128],
                            S[:, c, 2 * g:2 * g + 2, :],
                            start=(c == 0), stop=(c == 3))
                    nc.scalar.copy(wsel[:, m, 2 * g:2 * g + 2, :], pw[:])

            # ---------------- phase C: conv on selected channels ----------------
            for s in range(BS):
                xx = sbxx.tile([128, HP, HP], F32R, tag="xx")
                nc.gpsimd.memset(xx[:].bitcast(F32), 0.0)
                # lower half: x_pad rows 1..56 ; upper half: x_pad shifted one
                # row up (slot rr holds x_pad row rr+1)
                nc.gpsimd.dma_start(xx[0:64, 1:57, 1:57], xc.ap()[s])
                nc.gpsimd.dma_start(xx[64:128, 0:56, 1:57], xc.ap()[s])
                for tl in range(RT):
                    r0 = 1 + RPT * tl
                    pcv = ps_cv.tile([128, RPT, W], F32, tag="pcv")
                    for dx in range(3):
                        # dy0 (lower, rows r-1) + dy1 (upper slot r-1 = row r)
                        nc.tensor.matmul(
                            pcv[:], wsel[:, dx, s, :],
                            xx[:, r0 - 1:r0 + RPT - 1, dx:dx + W],
                            start=(dx == 0), stop=False)
                    for dx in range(3):
                        # dy2 = rows r+1: even tiles from lower half,
                        # odd tiles from upper half (slot r = row r+1)
                        if tl % 2 == 0:
                            nc.tensor.matmul(
                                pcv[:], wsel[0:64, 3 + dx, s, :],
                                xx[0:64, r0 + 1:r0 + RPT + 1, dx:dx + W],
                                start=False, stop=(dx == 2))
                        else:
                            nc.tensor.matmul(
                                pcv[:], wsel[64:128, 3 + dx, s, :],
                                xx[64:128, r0:r0 + RPT, dx:dx + W],
                                start=False, stop=(dx == 2))
                    ot = sbot.tile([128, RPT, W], F32, tag="ot")
                    nc.scalar.activation(ot[:], pcv[:], AFT.Identity,
                                         bias=bsel[:, s:s + 1], scale=1.0)
                    nc.sync.dma_start(
                        out.ap()[s, :, RPT * tl:RPT * tl + RPT, :], ot[:])

    nc.compile()
    return nc


def _prep_inputs(x, conv_w, conv_b, router_w, router_b):
    x = np.asarray(x, dtype=np.float32)
    conv_w = np.asarray(conv_w, dtype=np.float32)
    conv_b = np.asarray(conv_b, dtype=np.float32)
    router_w = np.asarray(router_w, dtype=np.float32)
    router_b = np.asarray(router_b, dtype=np.float32)

    x_flat = x.reshape(B, IN_DIM)
    xK = x_flat.reshape(B, IN_DIM // 128, 128)          # [s, K, p]
    rwT = np.ascontiguousarray(
        router_w.reshape(CH, IN_DIM // 128, 128).transpose(1, 2, 0))  # [K,p,co]

    w4 = conv_w.reshape(CH, CIN, 3, 3)
    wam = np.zeros((CH, 768), np.float32)
    for t in range(3):
        wam[:, t * 128:t * 128 + 64] = w4[:, :, 0, t]        # dy0
        wam[:, t * 128 + 64:t * 128 + 128] = w4[:, :, 1, t]  # dy1
        wam[:, (3 + t) * 128:(3 + t) * 128 + 64] = w4[:, :, 2, t]
        wam[:, (3 + t) * 128 + 64:(3 + t) * 128 + 128] = w4[:, :, 2, t]
    wa_dev = np.ascontiguousarray(wam.reshape(4, 128, 768))
    cb_dev = np.ascontiguousarray(conv_b.reshape(4, 128, 1))
    rb_dev = np.ascontiguousarray(
        np.broadcast_to(router_b[None, :], (BS, CH)))
    eye8 = np.eye(8, dtype=np.float32)
    iotaj = np.ascontiguousarray(
        np.broadcast_to(np.arange(128, dtype=np.float32)[None, :], (128, 128)))

    in_maps = []
    for r in range(NCORES):
        ks = slice(KC * r, KC * (r + 1))
        in_maps.append({
            "rw": np.ascontiguousarray(rwT[ks]),
            "xr": np.ascontiguousarray(xK[:, ks, :].transpose(2, 1, 0)),
            "xc": np.ascontiguousarray(x[BS * r:BS * (r + 1)]),
            "wa": wa_dev, "cb": cb_dev, "rb": rb_dev,
            "eye8": eye8, "iotaj": iotaj,
        })
    return in_maps


_NC_CACHE = None


def kernel(x, conv_w, conv_b, router_w, router_b):
    global _NC_CACHE
    if _NC_CACHE is None:
        _NC_CACHE = build_nc()
    nc = _NC_CACHE
    in_maps = _prep_inputs(x, conv_w, conv_b, router_w, router_b)
    res = run_bass_kernel_spmd(nc, in_maps, core_ids=list(range(NCORES)))
    return np.concatenate([res.results[r]["out"] for r in range(NCORES)], axis=0)


# revision 11
# speedup vs baseline: 126.9541x; 126.9541x over previous
"""MoELayer Trainium2 kernel (8 NeuronCores, SPMD).

Strategy:
  - Router matmul row-sharded over in_dim: each core computes partial scores
    for ALL 64 samples over its 25088-wide slice (fp32, exact), then a
    ReduceScatter(add) hands each core the final scores of its own 8 samples.
  - Exact top-128 per sample via bit-bisection on |scores| (int32 view of
    fp32 is order-isomorphic for non-negative floats), with jax.top_k tie
    semantics (lowest index wins) via an equality-cumsum pass.
  - Per-sample one-hot selection matrix S [512, 128] built on DVE; conv
    weights gathered as w_sel = wa.T @ S with float32r matmuls (values are
    0/1 so S is exact; weights round to f32r ~13-bit mantissa).
  - 3x3 conv on the 128 selected channels only (4x compute saving) in
    float32r: "double image" SBUF layout xx = [x_pad ; x_pad shifted one row]
    so (dy=0, dy=1) pack into one K=128 matmul; dy=2 runs as K=64 matmuls
    alternating between the two partition halves (row-tiling overlap).
  - PSUM drained by ScalarE with fused per-channel bias add.

Batch is data-parallel: core r owns samples [8r, 8r+8).
"""
import numpy as np

import concourse.bacc as bacc
import concourse.bass as bass
import concourse.mybir as mybir
import concourse.tile as tile
from concourse.bass_utils import run_bass_kernel_spmd

F32 = mybir.dt.float32
F32R = mybir.dt.float32r
I32 = mybir.dt.int32
OP = mybir.AluOpType
AFT = mybir.ActivationFunctionType

B, CIN, H, W = 64, 64, 56, 56
COUT, NEXP = 128, 4
CH = NEXP * COUT            # 512
IN_DIM = CIN * H * W        # 200704
NCORES = 8
BS = B // NCORES            # 8 samples per core
KC = IN_DIM // NCORES // 128  # 196 k-chunks of 128 per core
HP = H + 2                  # 58 padded
RT = 8                      # row-tiles per sample (7 output rows each)
RPT = H // RT               # 7 rows per tile


def build_nc(phase="full", num_devices=NCORES, skip_cc=False):
    nc = bacc.Bacc("TRN2", target_bir_lowering=False, debug=False,
                   num_devices=num_devices)

    rw = nc.dram_tensor("rw", [KC, 128, CH], F32, kind="ExternalInput")
    xr = nc.dram_tensor("xr", [128, KC, B], F32, kind="ExternalInput")
    xc = nc.dram_tensor("xc", [BS, CIN, H, W], F32, kind="ExternalInput")
    wa = nc.dram_tensor("wa", [4, 128, 768], F32, kind="ExternalInput")
    cb = nc.dram_tensor("cb", [4, 128, 1], F32, kind="ExternalInput")
    rb = nc.dram_tensor("rb", [BS, CH], F32, kind="ExternalInput")
    eye8 = nc.dram_tensor("eye8", [8, 8], F32, kind="ExternalInput")
    iotaj = nc.dram_tensor("iotaj", [128, 128], F32, kind="ExternalInput")
    out = nc.dram_tensor("out", [BS, COUT, H, W], F32, kind="ExternalOutput")

    with tile.TileContext(nc) as tc:
        with (
            tc.tile_pool(name="sb", bufs=1) as sb,
            tc.tile_pool(name="sbrw", bufs=8) as sbrw,
            tc.tile_pool(name="sbxx", bufs=2) as sbxx,
            tc.tile_pool(name="sbot", bufs=3) as sbot,
            tc.tile_pool(name="dram", bufs=1, space="DRAM") as dram,
            tc.tile_pool(name="ps_sc", bufs=1, space="PSUM") as ps_sc,
            tc.tile_pool(name="ps_tr", bufs=1, space="PSUM") as ps_tr,
            tc.tile_pool(name="ps_ws", bufs=2, space="PSUM") as ps_ws,
            tc.tile_pool(name="ps_cv", bufs=3, space="PSUM") as ps_cv,
        ):
            # ---------------- constants / static loads ----------------
            eyet = sb.tile([8, 8], F32, tag="eye")
            nc.sync.dma_start(eyet[:], eye8.ap())
            iott = sb.tile([128, 128], F32, tag="iot")
            nc.sync.dma_start(iott[:], iotaj.ap())
            rbt = sb.tile([BS, CH], F32, tag="rb")
            nc.sync.dma_start(rbt[:], rb.ap())
            wat = sb.tile([128, 4, 768], F32R, tag="wa")
            for c in range(4):
                nc.gpsimd.dma_start(wat[:, c, :], wa.ap()[c])
            cbt = sb.tile([128, 4, 1], F32, tag="cb")
            for c in range(4):
                nc.sync.dma_start(cbt[:, c, :], cb.ap()[c])

            def stash(ap2d, rows):
                """debug drain of a [rows, F] 2D AP into `out`."""
                f = ap2d.free_size()
                cwid = max(1, f // 16)
                nc.sync.dma_start(
                    out.ap()[0, 0:rows, 0:f // cwid, 0:cwid],
                    ap2d.rearrange("p (a c) -> p a c", c=cwid))

            # ---------------- phase R: router partial scores ----------------
            if phase != "null":
                xrt = sb.tile([128, KC, B], F32, tag="xr")
                nc.sync.dma_start(xrt[:], xr.ap())
                psc = ps_sc.tile([B, CH], F32, tag="psc")
                for k in range(KC):
                    rwk = sbrw.tile([128, CH], F32, tag="rwk")
                    nc.sync.dma_start(rwk[:], rw.ap()[k])
                    nc.tensor.matmul(psc[:], xrt[:, k, :], rwk[:],
                                     start=(k == 0), stop=(k == KC - 1))
                scp = sb.tile([B, CH], F32, tag="scp")
                nc.vector.tensor_copy(scp[:], psc[:])

            if phase == "null":
                nulltile = sb.tile([8, CH], F32, tag="nul")
                nc.sync.dma_start(nulltile[:], rb.ap())
                stash(nulltile[:], 8)
            if phase == "router":
                stash(scp[0:64, :], 64)

            if phase in ("rs", "topk", "wsel", "full", "timing"):
                scf = sb.tile([BS, CH], F32, tag="scf")
                if phase == "timing" or skip_cc:
                    # cost-model variant: skip the collective (~+12us on HW)
                    nc.vector.tensor_copy(scf[:], scp[0:BS, :])
                else:
                    rs_in = dram.tile([B, CH], F32)
                    rs_out = dram.tile([BS, CH], F32)
                    nc.sync.dma_start(rs_in[:], scp[:])
                    nc.gpsimd.collective_compute(
                        "ReduceScatter", OP.add,
                        replica_groups=[list(range(NCORES))],
                        ins=[rs_in.opt()], outs=[rs_out.opt()],
                    )
                    nc.sync.dma_start(scf[:], rs_out[:])
                nc.vector.tensor_tensor(scf[:], scf[:], rbt[:], OP.add)
            if phase == "rs":
                stash(scf[:], BS)

            if phase in ("topk", "wsel", "full", "timing"):
                # ---------------- phase T: exact top-128 ----------------
                sa = sb.tile([BS, CH], F32, tag="sa")
                nc.scalar.activation(sa[:], scf[:], AFT.Abs)
                lo = sb.tile([BS, 1], I32, tag="lo")
                nc.vector.memset(lo[:], 0)
                cand = sb.tile([BS, 1], I32, tag="cand")
                msk = sb.tile([BS, CH], F32, tag="msk")
                cnt = sb.tile([BS, 1], F32, tag="cnt")
                flag = sb.tile([BS, 1], F32, tag="flag")
                stpi = sb.tile([BS, 1], I32, tag="stpi")
                for b in range(30, -1, -1):
                    nc.vector.tensor_scalar(cand[:], lo[:], (1 << b), None,
                                            OP.add)
                    nc.vector.tensor_scalar(msk[:], sa[:],
                                            cand[:].bitcast(F32),
                                            None, OP.is_ge, OP.add,
                                            accum_out=cnt[:])
                    nc.vector.tensor_scalar(flag[:], cnt[:], float(COUT),
                                            None, OP.is_ge)
                    nc.vector.tensor_scalar(flag[:], flag[:], float(1 << b),
                                            None, OP.mult)
                    nc.vector.tensor_copy(stpi[:], flag[:])
                    nc.vector.tensor_tensor(lo[:], lo[:], stpi[:], OP.add)
                # lo == bits of the 128th largest |score| per sample
                mgt = sb.tile([BS, CH], F32, tag="mgt")
                ngt = sb.tile([BS, 1], F32, tag="ngt")
                nc.vector.tensor_scalar(mgt[:], sa[:], lo[:].bitcast(F32),
                                        None, OP.is_gt, OP.add,
                                        accum_out=ngt[:])
                meq = sb.tile([BS, CH], F32, tag="meq")
                nc.vector.tensor_scalar(meq[:], sa[:], lo[:].bitcast(F32),
                                        None, OP.is_equal)
                need = sb.tile([BS, 1], F32, tag="need")
                nc.vector.tensor_scalar(need[:], ngt[:], -1.0, None, OP.mult)
                nc.vector.tensor_scalar(need[:], need[:], float(COUT), None,
                                        OP.add)
                zf = sb.tile([BS, CH], F32, tag="zf")
                nc.vector.memset(zf[:], 0.0)
                cume = sb.tile([BS, CH], F32, tag="cume")
                nc.vector.tensor_tensor_scan(cume[:], meq[:], zf[:], 0.0,
                                             OP.add, OP.add)
                keep = sb.tile([BS, CH], F32, tag="keep")
                nc.vector.tensor_scalar(keep[:], cume[:], need[:], None,
                                        OP.is_le)
                nc.vector.tensor_tensor(keep[:], keep[:], meq[:], OP.mult)
                nc.vector.tensor_tensor(msk[:], mgt[:], keep[:], OP.add)
                cum = sb.tile([BS, CH], F32, tag="cum")
                nc.vector.tensor_tensor_scan(cum[:], msk[:], zf[:], 0.0,
                                             OP.add, OP.add)
                pos = sb.tile([BS, CH], F32, tag="pos")
                nc.vector.tensor_tensor(pos[:], cum[:], msk[:], OP.mult)
                nc.vector.tensor_scalar(pos[:], pos[:], -1.0, None, OP.add)

                posT = sb.tile([128, 4, BS], F32, tag="posT")
                for c in range(4):
                    ptr = ps_tr.tile([128, BS], F32, tag="ptr")
                    nc.tensor.transpose(ptr[:], pos[:, c * 128:(c + 1) * 128],
                                        eyet[:])
                    nc.vector.tensor_copy(posT[:, c, :], ptr[:])
            if phase == "topk":
                stash(pos[:], BS)

            if phase in ("wsel", "full", "timing"):
                # ------------ phase S: selection matrices + weight gather ----
                S = sb.tile([128, 4, BS, 128], F32R, tag="S")
                S32 = sb.tile([128, 4, BS, 128], F32, tag="S32")
                for c in range(4):
                    for s in range(BS):
                        nc.vector.tensor_scalar(S[:, c, s, :], iott[:],
                                                posT[:, c, s:s + 1], None,
                                                OP.is_equal)
                        nc.vector.tensor_scalar(S32[:, c, s, :], iott[:],
                                                posT[:, c, s:s + 1], None,
                                                OP.is_equal)
                # per-sample selected bias [128j, 1]
                bsel = sb.tile([128, BS], F32, tag="bsel")
                for s in range(BS):
                    pb = ps_tr.tile([128, 1], F32, tag="pb")
                    for c in range(4):
                        nc.tensor.matmul(pb[:], S32[:, c, s, :], cbt[:, c, :],
                                         start=(c == 0), stop=(c == 3))
                    nc.scalar.copy(bsel[:, s:s + 1], pb[:])
                # gathered weights wsel[m-chunk][s][j]
                wsel = sb.tile([128, 6, BS, 128], F32R, tag="wsel")
                for g in range(4):          # groups of 2 samples (N=256)
                    for m in range(6):
                        pw = ps_ws.tile([128, 2, 128], F32, tag="pw")
                        for c in range(4):
                            nc.tensor.matmul(
                                pw[:], wat[:, c, m * 128:(m + 1) * 128],
                                S[:, c, 2 * g:2 * g + 2, :],
                                start=(c == 0), stop=(c == 3))
                        nc.scalar.copy(wsel[:, m, 2 * g:2 * g + 2, :], pw[:])
            if phase == "wsel":
                stash(wsel[:, 0, 0, :].bitcast(F32), 128)

            if phase in ("full", "timing"):
                # ------------ phase C: conv on selected channels ------------
                for s in range(BS):
                    xx = sbxx.tile([128, HP, HP], F32R, tag="xx")
                    xxf = xx[:].bitcast(F32)
                    nc.gpsimd.memset(xxf[:, :, 0:1], 0.0)
                    nc.gpsimd.memset(xxf[:, :, 57:58], 0.0)
                    nc.gpsimd.memset(xxf[0:64, 0:1, 1:57], 0.0)
                    nc.gpsimd.memset(xxf[0:64, 57:58, 1:57], 0.0)
                    nc.gpsimd.memset(xxf[64:128, 56:58, 1:57], 0.0)
                    # lower: x_pad rows; upper: x_pad shifted one row up
                    nc.gpsimd.dma_start(xx[0:64, 1:57, 1:57], xc.ap()[s])
                    nc.gpsimd.dma_start(xx[64:128, 0:56, 1:57], xc.ap()[s])
                    for tl in range(RT):
                        r0 = 1 + RPT * tl
                        pcv = ps_cv.tile([128, RPT, W], F32, tag="pcv")
                        for dx in range(3):
                            # dy0 (lower, rows r-1) + dy1 (upper slot r-1)
                            nc.tensor.matmul(
                                pcv[:], wsel[:, dx, s, :],
                                xx[:, r0 - 1:r0 + RPT - 1, dx:dx + W],
                                start=(dx == 0), stop=False)
                        for dx in range(3):
                            # dy2 = rows r+1
                            if tl % 2 == 0:
                                nc.tensor.matmul(
                                    pcv[:], wsel[0:64, 3 + dx, s, :],
                                    xx[0:64, r0 + 1:r0 + RPT + 1, dx:dx + W],
                                    start=False, stop=(dx == 2))
                            else:
                                nc.tensor.matmul(
                                    pcv[:], wsel[64:128, 3 + dx, s, :],
                                    xx[64:128, r0:r0 + RPT, dx:dx + W],
                                    start=False, stop=(dx == 2))
                        ot = sbot.tile([128, RPT, W], F32, tag="ot")
                        nc.scalar.activation(ot[:], pcv[:], AFT.Identity,
                                             bias=bsel[:, s:s + 1], scale=1.0)
                        nc.sync.dma_start(
                            out.ap()[s, :, RPT * tl:RPT * tl + RPT, :], ot[:])

    nc.compile()
    return nc


def _prep_inputs(x, conv_w, conv_b, router_w, router_b):
    x = np.asarray(x, dtype=np.float32)
    conv_w = np.asarray(conv_w, dtype=np.float32)
    conv_b = np.asarray(conv_b, dtype=np.float32)
    router_w = np.asarray(router_w, dtype=np.float32)
    router_b = np.asarray(router_b, dtype=np.float32)

    x_flat = x.reshape(B, IN_DIM)
    xK = x_flat.reshape(B, IN_DIM // 128, 128)          # [s, K, p]
    rwT = np.ascontiguousarray(
        router_w.reshape(CH, IN_DIM // 128, 128).transpose(1, 2, 0))  # [K,p,co]

    w4 = conv_w.reshape(CH, CIN, 3, 3)
    wam = np.zeros((CH, 768), np.float32)
    for t in range(3):
        wam[:, t * 128:t * 128 + 64] = w4[:, :, 0, t]        # dy0
        wam[:, t * 128 + 64:t * 128 + 128] = w4[:, :, 1, t]  # dy1
        wam[:, (3 + t) * 128:(3 + t) * 128 + 64] = w4[:, :, 2, t]
        wam[:, (3 + t) * 128 + 64:(3 + t) * 128 + 128] = w4[:, :, 2, t]
    wa_dev = np.ascontiguousarray(wam.reshape(4, 128, 768))
    cb_dev = np.ascontiguousarray(conv_b.reshape(4, 128, 1))
    rb_dev = np.ascontiguousarray(
        np.broadcast_to(router_b[None, :], (BS, CH)))
    eye8 = np.eye(8, dtype=np.float32)
    iotaj = np.ascontiguousarray(
        np.broadcast_to(np.arange(128, dtype=np.float32)[None, :], (128, 128)))

    in_maps = []
    for r in range(NCORES):
        ks = slice(KC * r, KC * (r + 1))
        in_maps.append({
            "rw": np.ascontiguousarray(rwT[ks]),
            "xr": np.ascontiguousarray(xK[:, ks, :].transpose(2, 1, 0)),
            "xc": np.ascontiguousarray(x[BS * r:BS * (r + 1)]),
            "wa": wa_dev, "cb": cb_dev, "rb": rb_dev,
            "eye8": eye8, "iotaj": iotaj,
        })
    return in_maps


_NC_CACHE = None


def kernel(x, conv_w, conv_b, router_w, router_b):
    global _NC_CACHE
    if _NC_CACHE is None:
        _NC_CACHE = build_nc()
    nc = _NC_CACHE
    in_maps = _prep_inputs(x, conv_w, conv_b, router_w, router_b)
    res = run_bass_kernel_spmd(nc, in_maps, core_ids=list(range(NCORES)))
    return np.concatenate(
        [res.results[r]["out"] for r in range(NCORES)], axis=0)


# revision 28
# speedup vs baseline: 149.5319x; 1.1778x over previous
"""MoELayer Trainium2 kernel (8 NeuronCores, SPMD).

Strategy:
  - Router matmul row-sharded over in_dim: each core computes partial scores
    for ALL 64 samples over its 25088-wide slice (fp32, exact), then a
    ReduceScatter(add) hands each core the final scores of its own 8 samples.
  - Exact top-128 per sample via bit-bisection on |scores| (int32 view of
    fp32 is order-isomorphic for non-negative floats), with jax.top_k tie
    semantics (lowest index wins) via an equality-cumsum pass.
  - Per-sample one-hot selection matrix S [512, 128] built on DVE; conv
    weights gathered as w_sel = wa.T @ S with float32r matmuls (values are
    0/1 so S is exact; weights round to f32r ~13-bit mantissa).
  - 3x3 conv on the 128 selected channels only (4x compute saving) in
    float32r: "double image" SBUF layout xx = [x_pad ; x_pad shifted one row]
    so (dy=0, dy=1) pack into one K=128 matmul; dy=2 runs as K=64 matmuls
    alternating between the two partition halves (row-tiling overlap).
  - PSUM drained by ScalarE with fused per-channel bias add.

Batch is data-parallel: core r owns samples [8r, 8r+8).
"""
import numpy as np

import concourse.bacc as bacc
import concourse.bass as bass
import concourse.mybir as mybir
import concourse.tile as tile
from concourse.bass_utils import run_bass_kernel_spmd

F32 = mybir.dt.float32
F32R = mybir.dt.float32r
I32 = mybir.dt.int32
OP = mybir.AluOpType
AFT = mybir.ActivationFunctionType

B, CIN, H, W = 64, 64, 56, 56
COUT, NEXP = 128, 4
CH = NEXP * COUT            # 512
IN_DIM = CIN * H * W        # 200704
NCORES = 8
BS = B // NCORES            # 8 samples per core
KC = IN_DIM // NCORES // 128  # 196 k-chunks of 128 per core
HP = H + 2                  # 58 padded
RT = 7                      # row-tiles per sample (8 output rows each)
RPT = H // RT               # 8 rows per tile


def build_nc(phase="full", num_devices=NCORES, skip_cc=False):
    nc = bacc.Bacc("TRN2", target_bir_lowering=False, debug=False,
                   num_devices=num_devices)

    rw = nc.dram_tensor("rw", [KC, 128, CH], F32, kind="ExternalInput")
    xr = nc.dram_tensor("xr", [128, KC, B], F32, kind="ExternalInput")
    xc = nc.dram_tensor("xc", [BS, CIN, H, W], F32, kind="ExternalInput")
    wa = nc.dram_tensor("wa", [4, 128, 768], F32, kind="ExternalInput")
    cb = nc.dram_tensor("cb", [4, 128, 1], F32, kind="ExternalInput")
    rb = nc.dram_tensor("rb", [BS, CH], F32, kind="ExternalInput")
    eye8 = nc.dram_tensor("eye8", [8, 8], F32, kind="ExternalInput")
    iotaj = nc.dram_tensor("iotaj", [128, 128], F32, kind="ExternalInput")
    out = nc.dram_tensor("out", [BS, COUT, H, W], F32, kind="ExternalOutput")

    with tile.TileContext(nc) as tc:
        with (
            tc.tile_pool(name="sb", bufs=1) as sb,
            tc.tile_pool(name="sbrw", bufs=6) as sbrw,
            tc.tile_pool(name="sbxx", bufs=2) as sbxx,
            tc.tile_pool(name="sbxs", bufs=1) as sbxs,
            tc.tile_pool(name="sbot", bufs=3) as sbot,
            tc.tile_pool(name="dram", bufs=1, space="DRAM") as dram,
            tc.tile_pool(name="ps_sc", bufs=2, space="PSUM") as ps_sc,
            tc.tile_pool(name="ps_tr", bufs=1, space="PSUM") as ps_tr,
            tc.tile_pool(name="ps_ws", bufs=2, space="PSUM") as ps_ws,
            tc.tile_pool(name="ps_cv", bufs=2, space="PSUM") as ps_cv,
        ):
            # ---------------- constants / static loads ----------------
            eyet = sb.tile([8, 8], F32, tag="eye")
            nc.sync.dma_start(eyet[:], eye8.ap())
            iott = sb.tile([128, 128], F32, tag="iot")
            nc.sync.dma_start(iott[:], iotaj.ap())
            rbt = sb.tile([BS, CH], F32, tag="rb")
            nc.sync.dma_start(rbt[:], rb.ap())
            wat = sb.tile([128, 4, 768], F32R, tag="wa")
            for c in range(4):
                nc.gpsimd.dma_start(wat[:, c, :], wa.ap()[c])
            cbt = sb.tile([128, 4, 1], F32, tag="cb")
            for c in range(4):
                nc.sync.dma_start(cbt[:, c, :], cb.ap()[c])

            def stash(ap2d, rows):
                """debug drain of a [rows, F] 2D AP into `out`."""
                f = ap2d.free_size()
                cwid = max(1, f // 16)
                nc.sync.dma_start(
                    out.ap()[0, 0:rows, 0:f // cwid, 0:cwid],
                    ap2d.rearrange("p (a c) -> p a c", c=cwid))

            # ---------------- phase R: router partial scores ----------------
            if phase != "null":
                xrt = sb.tile([128, KC, B], F32, tag="xr")
                for kk in range(0, KC, 14):
                    nc.scalar.dma_start(xrt[:, kk:kk + 14, :],
                                        xr.ap()[:, kk:kk + 14, :])
                psc0 = ps_sc.tile([B, CH], F32, tag="psc")
                psc1 = ps_sc.tile([B, CH], F32, tag="psc")
                for k in range(KC):
                    rwk = sbrw.tile([128, CH], F32, tag="rwk")
                    nc.sync.dma_start(rwk[:], rw.ap()[k])
                    nc.tensor.matmul(psc0 if k % 2 == 0 else psc1,
                                     xrt[:, k, :], rwk[:],
                                     start=(k < 2), stop=(k >= KC - 2))
                scp = sb.tile([B, CH], F32, tag="scp")
                nc.vector.tensor_copy(scp[:], psc0[:])
                nc.vector.tensor_tensor(scp[:], scp[:], psc1[:], OP.add)

            if phase == "null":
                nulltile = sb.tile([8, CH], F32, tag="nul")
                nc.sync.dma_start(nulltile[:], rb.ap())
                stash(nulltile[:], 8)
            if phase == "router":
                stash(scp[0:64, :], 64)

            if phase in ("rs", "topk", "wsel", "full", "timing"):
                scf = sb.tile([BS, CH], F32, tag="scf")
                if phase == "timing" or skip_cc:
                    # cost-model variant: skip the collective (~+12us on HW)
                    nc.vector.tensor_copy(scf[:], scp[0:BS, :])
                else:
                    rs_in = dram.tile([B, CH], F32)
                    rs_out = dram.tile([BS, CH], F32)
                    nc.sync.dma_start(rs_in[:], scp[:])
                    nc.gpsimd.collective_compute(
                        "ReduceScatter", OP.add,
                        replica_groups=[list(range(NCORES))],
                        ins=[rs_in.opt()], outs=[rs_out.opt()],
                    )
                    nc.sync.dma_start(scf[:], rs_out[:])
                nc.vector.tensor_tensor(scf[:], scf[:], rbt[:], OP.add)
            if phase == "rs":
                stash(scf[:], BS)

            if phase in ("topk", "wsel", "full", "timing"):
                # ---------------- phase T: exact top-128 ----------------
                sa = sb.tile([BS, CH], F32, tag="sa")
                nc.scalar.activation(sa[:], scf[:], AFT.Abs)
                lo = sb.tile([BS, 1], I32, tag="lo")
                nc.vector.memset(lo[:], 0)
                cand = sb.tile([BS, 1], I32, tag="cand")
                msk = sb.tile([BS, CH], F32, tag="msk")
                cnt = sb.tile([BS, 1], F32, tag="cnt")
                flag = sb.tile([BS, 1], F32, tag="flag")
                stpi = sb.tile([BS, 1], I32, tag="stpi")
                for b in range(30, -1, -1):
                    nc.vector.tensor_scalar(cand[:], lo[:], (1 << b), None,
                                            OP.add)
                    nc.vector.tensor_scalar(msk[:], sa[:],
                                            cand[:].bitcast(F32),
                                            None, OP.is_ge, OP.add,
                                            accum_out=cnt[:])
                    nc.vector.tensor_scalar(flag[:], cnt[:], float(COUT),
                                            float(1 << b), OP.is_ge, OP.mult)
                    nc.vector.tensor_copy(stpi[:], flag[:])
                    nc.vector.tensor_tensor(lo[:], lo[:], stpi[:], OP.add)
                # lo == bits of the 128th largest |score| per sample
                mgt = sb.tile([BS, CH], F32, tag="mgt")
                ngt = sb.tile([BS, 1], F32, tag="ngt")
                nc.vector.tensor_scalar(mgt[:], sa[:], lo[:].bitcast(F32),
                                        None, OP.is_gt, OP.add,
                                        accum_out=ngt[:])
                meq = sb.tile([BS, CH], F32, tag="meq")
                nc.vector.tensor_scalar(meq[:], sa[:], lo[:].bitcast(F32),
                                        None, OP.is_equal)
                need = sb.tile([BS, 1], F32, tag="need")
                nc.vector.tensor_scalar(need[:], ngt[:], -1.0, None, OP.mult)
                nc.vector.tensor_scalar(need[:], need[:], float(COUT), None,
                                        OP.add)
                zf = sb.tile([BS, CH], F32, tag="zf")
                nc.vector.memset(zf[:], 0.0)
                cume = sb.tile([BS, CH], F32, tag="cume")
                nc.vector.tensor_tensor_scan(cume[:], meq[:], zf[:], 0.0,
                                             OP.add, OP.add)
                keep = sb.tile([BS, CH], F32, tag="keep")
                nc.vector.tensor_scalar(keep[:], cume[:], need[:], None,
                                        OP.is_le)
                nc.vector.tensor_tensor(keep[:], keep[:], meq[:], OP.mult)
                nc.vector.tensor_tensor(msk[:], mgt[:], keep[:], OP.add)
                cum = sb.tile([BS, CH], F32, tag="cum")
                nc.vector.tensor_tensor_scan(cum[:], msk[:], zf[:], 0.0,
                                             OP.add, OP.add)
                pos = sb.tile([BS, CH], F32, tag="pos")
                nc.vector.tensor_tensor(pos[:], cum[:], msk[:], OP.mult)
                nc.vector.tensor_scalar(pos[:], pos[:], -1.0, None, OP.add)

                posT = sb.tile([128, 4, BS], F32, tag="posT")
                for c in range(4):
                    ptr = ps_tr.tile([128, BS], F32, tag="ptr")
                    nc.tensor.transpose(ptr[:], pos[:, c * 128:(c + 1) * 128],
                                        eyet[:])
                    nc.vector.tensor_copy(posT[:, c, :], ptr[:])
            if phase == "topk":
                stash(pos[:], BS)

            if phase in ("wsel", "full", "timing"):
                # ------------ phase S: selection matrices + weight gather ----
                S = sb.tile([128, 4, BS, 128], F32R, tag="S")
                S32 = sb.tile([128, 4, BS, 128], F32, tag="S32")
                for c in range(4):
                    for s in range(BS):
                        nc.vector.tensor_scalar(S32[:, c, s, :], iott[:],
                                                posT[:, c, s:s + 1], None,
                                                OP.is_equal)
                nc.vector.tensor_copy(S[:], S32[:])
                # per-sample selected bias [128j, 1]
                bsel = sb.tile([128, BS], F32, tag="bsel")
                for s in range(BS):
                    pb = ps_tr.tile([128, 1], F32, tag="pb")
                    for c in range(4):
                        nc.tensor.matmul(pb[:], S32[:, c, s, :], cbt[:, c, :],
                                         start=(c == 0), stop=(c == 3))
                    nc.scalar.copy(bsel[:, s:s + 1], pb[:])
                # gathered weights wsel[m-chunk][s][j]
                wsel = sb.tile([128, 6, BS, 128], F32R, tag="wsel")
                for g in range(2):          # groups of 4 samples (N=512)
                    for m in range(6):
                        pw = ps_ws.tile([128, 4, 128], F32, tag="pw")
                        for c in range(4):
                            nc.tensor.matmul(
                                pw[:], wat[:, c, m * 128:(m + 1) * 128],
                                S[:, c, 4 * g:4 * g + 4, :],
                                start=(c == 0), stop=(c == 3))
                        nc.scalar.copy(wsel[:, m, 4 * g:4 * g + 4, :], pw[:])
            if phase == "wsel":
                stash(wsel[:, 0, 0, :].bitcast(F32), 128)

            if phase in ("full", "timing"):
                # ------------ phase C: conv on selected channels ------------
                for s in range(BS):
                    xx = sbxx.tile([128, HP, HP], F32R, tag="xx")
                    xxf = xx[:].bitcast(F32)
                    nc.gpsimd.memset(xxf[:, :, 0:1], 0.0)
                    nc.gpsimd.memset(xxf[:, :, 57:58], 0.0)
                    nc.gpsimd.memset(xxf[0:64, 0:1, 1:57], 0.0)
                    nc.gpsimd.memset(xxf[0:64, 57:58, 1:57], 0.0)
                    nc.gpsimd.memset(xxf[64:128, 56:58, 1:57], 0.0)
                    # stage x via fast sync DMA, cast f32->f32r on DVE
                    xst = sbxs.tile([128, H, W], F32, tag="xst")
                    nc.sync.dma_start(xst[0:64, :, :], xc.ap()[s])
                    nc.sync.dma_start(xst[64:128, :, :], xc.ap()[s])
                    # lower: x_pad rows; upper: x_pad shifted one row up
                    nc.vector.tensor_copy(xx[0:64, 1:57, 1:57], xst[0:64, :, :])
                    nc.vector.tensor_copy(xx[64:128, 0:56, 1:57],
                                          xst[64:128, :, :])
                    for tl in range(RT):
                        r0 = 1 + RPT * tl
                        pcv = ps_cv.tile([128, RPT, W], F32, tag="pcv")
                        for dx in range(3):
                            # dy0 (lower, rows r-1) + dy1 (upper slot r-1)
                            nc.tensor.matmul(
                                pcv[:], wsel[:, dx, s, :],
                                xx[:, r0 - 1:r0 + RPT - 1, dx:dx + W],
                                start=(dx == 0), stop=False)
                        for dx in range(3):
                            # dy2 = rows r+1
                            if tl % 2 == 0:
                                nc.tensor.matmul(
                                    pcv[:], wsel[0:64, 3 + dx, s, :],
                                    xx[0:64, r0 + 1:r0 + RPT + 1, dx:dx + W],
                                    start=False, stop=(dx == 2))
                            else:
                                nc.tensor.matmul(
                                    pcv[:], wsel[64:128, 3 + dx, s, :],
                                    xx[64:128, r0:r0 + RPT, dx:dx + W],
                                    start=False, stop=(dx == 2))
                        ot = sbot.tile([128, RPT, W], F32, tag="ot")
                        nc.scalar.activation(ot[:], pcv[:], AFT.Identity,
                                             bias=bsel[:, s:s + 1], scale=1.0)
                        nc.sync.dma_start(
                            out.ap()[s, :, RPT * tl:RPT * tl + RPT, :], ot[:])

    nc.compile()
    return nc


def _prep_inputs(x, conv_w, conv_b, router_w, router_b):
    x = np.asarray(x, dtype=np.float32)
    conv_w = np.asarray(conv_w, dtype=np.float32)
    conv_b = np.asarray(conv_b, dtype=np.float32)
    router_w = np.asarray(router_w, dtype=np.float32)
    router_b = np.asarray(router_b, dtype=np.float32)

    x_flat = x.reshape(B, IN_DIM)
    xK = x_flat.reshape(B, IN_DIM // 128, 128)          # [s, K, p]
    rwT = np.ascontiguousarray(
        router_w.reshape(CH, IN_DIM // 128, 128).transpose(1, 2, 0))  # [K,p,co]

    w4 = conv_w.reshape(CH, CIN, 3, 3)
    wam = np.zeros((CH, 768), np.float32)
    for t in range(3):
        wam[:, t * 128:t * 128 + 64] = w4[:, :, 0, t]        # dy0
        wam[:, t * 128 + 64:t * 128 + 128] = w4[:, :, 1, t]  # dy1
        wam[:, (3 + t) * 128:(3 + t) * 128 + 64] = w4[:, :, 2, t]
        wam[:, (3 + t) * 128 + 64:(3 + t) * 128 + 128] = w4[:, :, 2, t]
    wa_dev = np.ascontiguousarray(wam.reshape(4, 128, 768))
    cb_dev = np.ascontiguousarray(conv_b.reshape(4, 128, 1))
    rb_dev = np.ascontiguousarray(
        np.broadcast_to(router_b[None, :], (BS, CH)))
    eye8 = np.eye(8, dtype=np.float32)
    iotaj = np.ascontiguousarray(
        np.broadcast_to(np.arange(128, dtype=np.float32)[None, :], (128, 128)))

    in_maps = []
    for r in range(NCORES):
        ks = slice(KC * r, KC * (r + 1))
        in_maps.append({
            "rw": np.ascontiguousarray(rwT[ks]),
            "xr": np.ascontiguousarray(xK[:, ks, :].transpose(2, 1, 0)),
            "xc": np.ascontiguousarray(x[BS * r:BS * (r + 1)]),
            "wa": wa_dev, "cb": cb_dev, "rb": rb_dev,
            "eye8": eye8, "iotaj": iotaj,
        })
    return in_maps


_NC_CACHE = None


def kernel(x, conv_w, conv_b, router_w, router_b):
    global _NC_CACHE
    if _NC_CACHE is None:
        _NC_CACHE = build_nc()
    nc = _NC_CACHE
    in_maps = _prep_inputs(x, conv_w, conv_b, router_w, router_b)
    res = run_bass_kernel_spmd(nc, in_maps, core_ids=list(range(NCORES)))
    return np.concatenate(
        [res.results[r]["out"] for r in range(NCORES)], axis=0)


# revision 33
# speedup vs baseline: 150.3337x; 1.0054x over previous
"""MoELayer Trainium2 kernel (8 NeuronCores, SPMD).

Strategy:
  - Router matmul row-sharded over in_dim: each core computes partial scores
    for ALL 64 samples over its 25088-wide slice (fp32, exact), then a
    ReduceScatter(add) hands each core the final scores of its own 8 samples.
  - Exact top-128 per sample via bit-bisection on |scores| (int32 view of
    fp32 is order-isomorphic for non-negative floats), with jax.top_k tie
    semantics (lowest index wins) via an equality-cumsum pass.
  - Per-sample one-hot selection matrix S [512, 128] built on DVE; conv
    weights gathered as w_sel = wa.T @ S with float32r matmuls (values are
    0/1 so S is exact; weights round to f32r ~13-bit mantissa).
  - 3x3 conv on the 128 selected channels only (4x compute saving) in
    float32r: "double image" SBUF layout xx = [x_pad ; x_pad shifted one row]
    so (dy=0, dy=1) pack into one K=128 matmul; dy=2 runs as K=64 matmuls
    alternating between the two partition halves (row-tiling overlap).
  - PSUM drained by ScalarE with fused per-channel bias add.

Batch is data-parallel: core r owns samples [8r, 8r+8).
"""
import numpy as np

import concourse.bacc as bacc
import concourse.bass as bass
import concourse.mybir as mybir
import concourse.tile as tile
from concourse.bass_utils import run_bass_kernel_spmd

F32 = mybir.dt.float32
F32R = mybir.dt.float32r
I32 = mybir.dt.int32
OP = mybir.AluOpType
AFT = mybir.ActivationFunctionType

B, CIN, H, W = 64, 64, 56, 56
COUT, NEXP = 128, 4
CH = NEXP * COUT            # 512
IN_DIM = CIN * H * W        # 200704
NCORES = 8
BS = B // NCORES            # 8 samples per core
KC = IN_DIM // NCORES // 128  # 196 k-chunks of 128 per core
HP = H + 2                  # 58 padded
RT = 7                      # row-tiles per sample (8 output rows each)
RPT = H // RT               # 8 rows per tile


def build_nc(phase="full", num_devices=NCORES, skip_cc=False):
    nc = bacc.Bacc("TRN2", target_bir_lowering=False, debug=False,
                   num_devices=num_devices)

    rw = nc.dram_tensor("rw", [KC, 128, CH], F32, kind="ExternalInput")
    xr = nc.dram_tensor("xr", [128, KC, B], F32, kind="ExternalInput")
    xc = nc.dram_tensor("xc", [BS, CIN, H, W], F32, kind="ExternalInput")
    wa = nc.dram_tensor("wa", [4, 128, 896], F32, kind="ExternalInput")
    cb = nc.dram_tensor("cb", [4, 128, 1], F32, kind="ExternalInput")
    rb = nc.dram_tensor("rb", [BS, CH], F32, kind="ExternalInput")
    eye8 = nc.dram_tensor("eye8", [8, 8], F32, kind="ExternalInput")
    iotaj = nc.dram_tensor("iotaj", [128, 128], F32, kind="ExternalInput")
    out = nc.dram_tensor("out", [BS, COUT, H, W], F32, kind="ExternalOutput")

    with tile.TileContext(nc) as tc:
        with (
            tc.tile_pool(name="sb", bufs=1) as sb,
            tc.tile_pool(name="sbrw", bufs=6) as sbrw,
            tc.tile_pool(name="sbxx", bufs=2) as sbxx,
            tc.tile_pool(name="sbxs", bufs=2) as sbxs,
            tc.tile_pool(name="sbot", bufs=3) as sbot,
            tc.tile_pool(name="dram", bufs=1, space="DRAM") as dram,
            tc.tile_pool(name="ps_sc", bufs=1, space="PSUM") as ps_sc,
            tc.tile_pool(name="ps_tr", bufs=1, space="PSUM") as ps_tr,
            tc.tile_pool(name="ps_ws", bufs=2, space="PSUM") as ps_ws,
            tc.tile_pool(name="ps_cv", bufs=3, space="PSUM") as ps_cv,
        ):
            # ---------------- constants / static loads ----------------
            eyet = sb.tile([8, 8], F32, tag="eye")
            nc.sync.dma_start(eyet[:], eye8.ap())
            iott = sb.tile([128, 128], F32, tag="iot")
            nc.sync.dma_start(iott[:], iotaj.ap())
            rbt = sb.tile([BS, CH], F32, tag="rb")
            nc.sync.dma_start(rbt[:], rb.ap())
            wat = sb.tile([128, 4, 896], F32R, tag="wa")
            for c in range(4):
                nc.gpsimd.dma_start(wat[:, c, :], wa.ap()[c])
            cbt = sb.tile([128, 4, 1], F32, tag="cb")
            for c in range(4):
                nc.sync.dma_start(cbt[:, c, :], cb.ap()[c])

            def stash(ap2d, rows):
                """debug drain of a [rows, F] 2D AP into `out`."""
                f = ap2d.free_size()
                cwid = max(1, f // 16)
                nc.sync.dma_start(
                    out.ap()[0, 0:rows, 0:f // cwid, 0:cwid],
                    ap2d.rearrange("p (a c) -> p a c", c=cwid))

            # ---------------- phase R: router partial scores ----------------
            if phase != "null":
                xrt = sb.tile([128, KC, B], F32, tag="xr")
                for kk in range(0, KC, 14):
                    nc.scalar.dma_start(xrt[:, kk:kk + 14, :],
                                        xr.ap()[:, kk:kk + 14, :])
                psc = ps_sc.tile([B, CH], F32, tag="psc")
                for k in range(KC):
                    rwk = sbrw.tile([128, CH], F32, tag="rwk")
                    nc.sync.dma_start(rwk[:], rw.ap()[k])
                    nc.tensor.matmul(psc[:], xrt[:, k, :], rwk[:],
                                     start=(k == 0), stop=(k == KC - 1))
                scp = sb.tile([B, CH], F32, tag="scp")
                nc.vector.tensor_copy(scp[:], psc[:])

            if phase == "null":
                nulltile = sb.tile([8, CH], F32, tag="nul")
                nc.sync.dma_start(nulltile[:], rb.ap())
                stash(nulltile[:], 8)
            if phase == "router":
                stash(scp[0:64, :], 64)

            if phase in ("rs", "topk", "wsel", "full", "timing"):
                scf = sb.tile([BS, CH], F32, tag="scf")
                if phase == "timing" or skip_cc:
                    # cost-model variant: skip the collective (~+12us on HW)
                    nc.vector.tensor_copy(scf[:], scp[0:BS, :])
                else:
                    rs_in = dram.tile([B, CH], F32)
                    rs_out = dram.tile([BS, CH], F32)
                    nc.sync.dma_start(rs_in[:], scp[:])
                    nc.gpsimd.collective_compute(
                        "ReduceScatter", OP.add,
                        replica_groups=[list(range(NCORES))],
                        ins=[rs_in.opt()], outs=[rs_out.opt()],
                    )
                    nc.sync.dma_start(scf[:], rs_out[:])
                nc.vector.tensor_tensor(scf[:], scf[:], rbt[:], OP.add)
            if phase == "rs":
                stash(scf[:], BS)

            if phase in ("topk", "wsel", "full", "timing"):
                # ---------------- phase T: exact top-128 ----------------
                sa = sb.tile([BS, CH], F32, tag="sa")
                nc.scalar.activation(sa[:], scf[:], AFT.Abs)
                lo = sb.tile([BS, 1], I32, tag="lo")
                nc.vector.memset(lo[:], 0)
                cand = sb.tile([BS, 1], I32, tag="cand")
                msk = sb.tile([BS, CH], F32, tag="msk")
                cnt = sb.tile([BS, 1], F32, tag="cnt")
                flag = sb.tile([BS, 1], F32, tag="flag")
                stpi = sb.tile([BS, 1], I32, tag="stpi")
                for b in range(30, -1, -1):
                    nc.vector.tensor_scalar(cand[:], lo[:], (1 << b), None,
                                            OP.add)
                    nc.vector.tensor_scalar(msk[:], sa[:],
                                            cand[:].bitcast(F32),
                                            None, OP.is_ge, OP.add,
                                            accum_out=cnt[:])
                    nc.vector.tensor_scalar(flag[:], cnt[:], float(COUT),
                                            float(1 << b), OP.is_ge, OP.mult)
                    nc.vector.tensor_copy(stpi[:], flag[:])
                    nc.vector.tensor_tensor(lo[:], lo[:], stpi[:], OP.add)
                # lo == bits of the 128th largest |score| per sample
                mgt = sb.tile([BS, CH], F32, tag="mgt")
                ngt = sb.tile([BS, 1], F32, tag="ngt")
                nc.vector.tensor_scalar(mgt[:], sa[:], lo[:].bitcast(F32),
                                        None, OP.is_gt, OP.add,
                                        accum_out=ngt[:])
                meq = sb.tile([BS, CH], F32, tag="meq")
                nc.vector.tensor_scalar(meq[:], sa[:], lo[:].bitcast(F32),
                                        None, OP.is_equal)
                need = sb.tile([BS, 1], F32, tag="need")
                nc.vector.tensor_scalar(need[:], ngt[:], -1.0, None, OP.mult)
                nc.vector.tensor_scalar(need[:], need[:], float(COUT), None,
                                        OP.add)
                zf = sb.tile([BS, CH], F32, tag="zf")
                nc.vector.memset(zf[:], 0.0)
                cume = sb.tile([BS, CH], F32, tag="cume")
                nc.vector.tensor_tensor_scan(cume[:], meq[:], zf[:], 0.0,
                                             OP.add, OP.add)
                keep = sb.tile([BS, CH], F32, tag="keep")
                nc.vector.tensor_scalar(keep[:], cume[:], need[:], None,
                                        OP.is_le)
                nc.vector.tensor_tensor(keep[:], keep[:], meq[:], OP.mult)
                nc.vector.tensor_tensor(msk[:], mgt[:], keep[:], OP.add)
                cum = sb.tile([BS, CH], F32, tag="cum")
                nc.vector.tensor_tensor_scan(cum[:], msk[:], zf[:], 0.0,
                                             OP.add, OP.add)
                pos = sb.tile([BS, CH], F32, tag="pos")
                nc.vector.tensor_tensor(pos[:], cum[:], msk[:], OP.mult)
                nc.vector.tensor_scalar(pos[:], pos[:], -1.0, None, OP.add)

                posT = sb.tile([128, 4, BS], F32, tag="posT")
                for c in range(4):
                    ptr = ps_tr.tile([128, BS], F32, tag="ptr")
                    nc.tensor.transpose(ptr[:], pos[:, c * 128:(c + 1) * 128],
                                        eyet[:])
                    nc.vector.tensor_copy(posT[:, c, :], ptr[:])
            if phase == "topk":
                stash(pos[:], BS)

            if phase in ("wsel", "full", "timing"):
                # ------------ phase S: selection matrices + weight gather ----
                S = sb.tile([128, 4, BS, 128], F32R, tag="S")
                for c in range(4):
                    for s in range(BS):
                        nc.vector.tensor_scalar(S[:, c, s, :], iott[:],
                                                posT[:, c, s:s + 1], None,
                                                OP.is_equal)
                # gathered weights wsel[m-chunk][s][j]; chunk 6 row 0 = bias
                wsel = sb.tile([128, 7, BS, 128], F32R, tag="wsel")
                for g in range(2):          # groups of 4 samples (N=512)
                    for m in range(7):
                        pw = ps_ws.tile([128, 4, 128], F32, tag="pw")
                        for c in range(4):
                            nc.tensor.matmul(
                                pw[:], wat[:, c, m * 128:(m + 1) * 128],
                                S[:, c, 4 * g:4 * g + 4, :],
                                start=(c == 0), stop=(c == 3))
                        nc.scalar.copy(wsel[:, m, 4 * g:4 * g + 4, :], pw[:])
                # bias row -> per-partition column via partition-scatter DMA
                bsel = sb.tile([128, BS], F32, tag="bsel")
                for s in range(BS):
                    nc.sync.dma_start(bsel[:, s:s + 1],
                                      wsel[0:1, 6, s, :].bitcast(F32))
            if phase == "wsel":
                stash(wsel[:, 0, 0, :].bitcast(F32), 128)

            if phase in ("full", "timing"):
                # ------------ phase C: conv on selected channels ------------
                for s in range(BS):
                    xx = sbxx.tile([128, HP, HP], F32R, tag="xx")
                    xxf = xx[:].bitcast(F32)
                    nc.gpsimd.memset(xxf[:, :, 0:1], 0.0)
                    nc.gpsimd.memset(xxf[:, :, 57:58], 0.0)
                    nc.gpsimd.memset(xxf[0:64, 0:1, 1:57], 0.0)
                    nc.gpsimd.memset(xxf[0:64, 57:58, 1:57], 0.0)
                    nc.gpsimd.memset(xxf[64:128, 56:58, 1:57], 0.0)
                    # stage x via fast sync DMA, cast f32->f32r on DVE
                    xst = sbxs.tile([128, H, W], F32, tag="xst")
                    nc.sync.dma_start(xst[0:64, :, :], xc.ap()[s])
                    nc.sync.dma_start(xst[64:128, :, :], xc.ap()[s])
                    # lower: x_pad rows; upper: x_pad shifted one row up
                    nc.vector.tensor_copy(xx[0:64, 1:57, 1:57], xst[0:64, :, :])
                    nc.vector.tensor_copy(xx[64:128, 0:56, 1:57],
                                          xst[64:128, :, :])
                    for tl in range(RT):
                        r0 = 1 + RPT * tl
                        pcv = ps_cv.tile([128, RPT, W], F32, tag="pcv")
                        for dx in range(3):
                            # dy0 (lower, rows r-1) + dy1 (upper slot r-1)
                            nc.tensor.matmul(
                                pcv[:], wsel[:, dx, s, :],
                                xx[:, r0 - 1:r0 + RPT - 1, dx:dx + W],
                                start=(dx == 0), stop=False)
                        for dx in range(3):
                            # dy2 = rows r+1
                            if tl % 2 == 0:
                                nc.tensor.matmul(
                                    pcv[:], wsel[0:64, 3 + dx, s, :],
                                    xx[0:64, r0 + 1:r0 + RPT + 1, dx:dx + W],
                                    start=False, stop=(dx == 2))
                            else:
                                nc.tensor.matmul(
                                    pcv[:], wsel[64:128, 3 + dx, s, :],
                                    xx[64:128, r0:r0 + RPT, dx:dx + W],
                                    start=False, stop=(dx == 2))
                        ot = sbot.tile([128, RPT, W], F32, tag="ot")
                        nc.scalar.activation(ot[:], pcv[:], AFT.Identity,
                                             bias=bsel[:, s:s + 1], scale=1.0)
                        nc.sync.dma_start(
                            out.ap()[s, :, RPT * tl:RPT * tl + RPT, :], ot[:])

    nc.compile()
    return nc


def _prep_inputs(x, conv_w, conv_b, router_w, router_b):
    x = np.asarray(x, dtype=np.float32)
    conv_w = np.asarray(conv_w, dtype=np.float32)
    conv_b = np.asarray(conv_b, dtype=np.float32)
    router_w = np.asarray(router_w, dtype=np.float32)
    router_b = np.asarray(router_b, dtype=np.float32)

    x_flat = x.reshape(B, IN_DIM)
    xK = x_flat.reshape(B, IN_DIM // 128, 128)          # [s, K, p]
    rwT = np.ascontiguousarray(
        router_w.reshape(CH, IN_DIM // 128, 128).transpose(1, 2, 0))  # [K,p,co]

    w4 = conv_w.reshape(CH, CIN, 3, 3)
    wam = np.zeros((CH, 896), np.float32)
    for t in range(3):
        wam[:, t * 128:t * 128 + 64] = w4[:, :, 0, t]        # dy0
        wam[:, t * 128 + 64:t * 128 + 128] = w4[:, :, 1, t]  # dy1
        wam[:, (3 + t) * 128:(3 + t) * 128 + 64] = w4[:, :, 2, t]
        wam[:, (3 + t) * 128 + 64:(3 + t) * 128 + 128] = w4[:, :, 2, t]
    wam[:, 768] = conv_b.reshape(CH)
    wa_dev = np.ascontiguousarray(wam.reshape(4, 128, 896))
    cb_dev = np.ascontiguousarray(conv_b.reshape(4, 128, 1))
    rb_dev = np.ascontiguousarray(
        np.broadcast_to(router_b[None, :], (BS, CH)))
    eye8 = np.eye(8, dtype=np.float32)
    iotaj = np.ascontiguousarray(
        np.broadcast_to(np.arange(128, dtype=np.float32)[None, :], (128, 128)))

    in_maps = []
    for r in range(NCORES):
        ks = slice(KC * r, KC * (r + 1))
        in_maps.append({
            "rw": np.ascontiguousarray(rwT[ks]),
            "xr": np.ascontiguousarray(xK[:, ks, :].transpose(2, 1, 0)),
            "xc": np.ascontiguousarray(x[BS * r:BS * (r + 1)]),
            "wa": wa_dev, "cb": cb_dev, "rb": rb_dev,
            "eye8": eye8, "iotaj": iotaj,
        })
    return in_maps


_NC_CACHE = None


def kernel(x, conv_w, conv_b, router_w, router_b):
    global _NC_CACHE
    if _NC_CACHE is None:
        _NC_CACHE = build_nc()
    nc = _NC_CACHE
    in_maps = _prep_inputs(x, conv_w, conv_b, router_w, router_b)
    res = run_bass_kernel_spmd(nc, in_maps, core_ids=list(range(NCORES)))
    return np.concatenate(
        [res.results[r]["out"] for r in range(NCORES)], axis=0)


# revision 36
# speedup vs baseline: 150.5242x; 1.0013x over previous
"""MoELayer Trainium2 kernel (8 NeuronCores, SPMD).

Strategy:
  - Router matmul row-sharded over in_dim: each core computes partial scores
    for ALL 64 samples over its 25088-wide slice (fp32, exact), then a
    ReduceScatter(add) hands each core the final scores of its own 8 samples.
  - Exact top-128 per sample via bit-bisection on |scores| (int32 view of
    fp32 is order-isomorphic for non-negative floats), with jax.top_k tie
    semantics (lowest index wins) via an equality-cumsum pass.
  - Per-sample one-hot selection matrix S [512, 128] built on DVE; conv
    weights gathered as w_sel = wa.T @ S with float32r matmuls (values are
    0/1 so S is exact; weights round to f32r ~13-bit mantissa).
  - 3x3 conv on the 128 selected channels only (4x compute saving) in
    float32r: "double image" SBUF layout xx = [x_pad ; x_pad shifted one row]
    so (dy=0, dy=1) pack into one K=128 matmul; dy=2 runs as K=64 matmuls
    alternating between the two partition halves (row-tiling overlap).
  - PSUM drained by ScalarE with fused per-channel bias add.

Batch is data-parallel: core r owns samples [8r, 8r+8).
"""
import numpy as np

import concourse.bacc as bacc
import concourse.bass as bass
import concourse.mybir as mybir
import concourse.tile as tile
from concourse.bass_utils import run_bass_kernel_spmd

F32 = mybir.dt.float32
F32R = mybir.dt.float32r
I32 = mybir.dt.int32
OP = mybir.AluOpType
AFT = mybir.ActivationFunctionType

B, CIN, H, W = 64, 64, 56, 56
COUT, NEXP = 128, 4
CH = NEXP * COUT            # 512
IN_DIM = CIN * H * W        # 200704
NCORES = 8
BS = B // NCORES            # 8 samples per core
KC = IN_DIM // NCORES // 128  # 196 k-chunks of 128 per core
HP = H + 2                  # 58 padded
RT = 7                      # row-tiles per sample (8 output rows each)
RPT = H // RT               # 8 rows per tile


def build_nc(phase="full", num_devices=NCORES, skip_cc=False):
    nc = bacc.Bacc("TRN2", target_bir_lowering=False, debug=False,
                   num_devices=num_devices)

    rw = nc.dram_tensor("rw", [KC, 128, CH], F32, kind="ExternalInput")
    xr = nc.dram_tensor("xr", [128, KC, B], F32, kind="ExternalInput")
    xc = nc.dram_tensor("xc", [BS, CIN, H, W], F32, kind="ExternalInput")
    wa = nc.dram_tensor("wa", [4, 128, 896], F32, kind="ExternalInput")
    cb = nc.dram_tensor("cb", [4, 128, 1], F32, kind="ExternalInput")
    rb = nc.dram_tensor("rb", [BS, CH], F32, kind="ExternalInput")
    eye8 = nc.dram_tensor("eye8", [8, 8], F32, kind="ExternalInput")
    iotaj = nc.dram_tensor("iotaj", [128, 128], F32, kind="ExternalInput")
    out = nc.dram_tensor("out", [BS, COUT, H, W], F32, kind="ExternalOutput")

    with tile.TileContext(nc) as tc:
        with (
            tc.tile_pool(name="sb", bufs=1) as sb,
            tc.tile_pool(name="sbrw", bufs=6) as sbrw,
            tc.tile_pool(name="sbxx", bufs=2) as sbxx,
            tc.tile_pool(name="sbxs", bufs=2) as sbxs,
            tc.tile_pool(name="sbot", bufs=3) as sbot,
            tc.tile_pool(name="dram", bufs=1, space="DRAM") as dram,
            tc.tile_pool(name="ps_sc", bufs=1, space="PSUM") as ps_sc,
            tc.tile_pool(name="ps_tr", bufs=1, space="PSUM") as ps_tr,
            tc.tile_pool(name="ps_ws", bufs=2, space="PSUM") as ps_ws,
            tc.tile_pool(name="ps_cv", bufs=3, space="PSUM") as ps_cv,
        ):
            # ---------------- constants / static loads ----------------
            eyet = sb.tile([8, 8], F32, tag="eye")
            nc.sync.dma_start(eyet[:], eye8.ap())
            iott = sb.tile([128, 128], F32, tag="iot")
            nc.sync.dma_start(iott[:], iotaj.ap())
            rbt = sb.tile([BS, CH], F32, tag="rb")
            nc.sync.dma_start(rbt[:], rb.ap())
            wat = sb.tile([128, 4, 896], F32R, tag="wa")
            for c in range(4):
                nc.gpsimd.dma_start(wat[:, c, :], wa.ap()[c])
            cbt = sb.tile([128, 4, 1], F32, tag="cb")
            for c in range(4):
                nc.sync.dma_start(cbt[:, c, :], cb.ap()[c])

            def stash(ap2d, rows):
                """debug drain of a [rows, F] 2D AP into `out`."""
                f = ap2d.free_size()
                cwid = max(1, f // 16)
                nc.sync.dma_start(
                    out.ap()[0, 0:rows, 0:f // cwid, 0:cwid],
                    ap2d.rearrange("p (a c) -> p a c", c=cwid))

            # ---------------- phase R: router partial scores ----------------
            if phase != "null":
                xrt = sb.tile([128, KC, B], F32, tag="xr")
                for kk in range(0, KC, 7):
                    nc.scalar.dma_start(xrt[:, kk:kk + 7, :],
                                        xr.ap()[:, kk:kk + 7, :])
                psc = ps_sc.tile([B, CH], F32, tag="psc")
                for k in range(KC):
                    rwk = sbrw.tile([128, CH], F32, tag="rwk")
                    nc.sync.dma_start(rwk[:], rw.ap()[k])
                    nc.tensor.matmul(psc[:], xrt[:, k, :], rwk[:],
                                     start=(k == 0), stop=(k == KC - 1))
                scp = sb.tile([B, CH], F32, tag="scp")
                nc.vector.tensor_copy(scp[:], psc[:])

            if phase == "null":
                nulltile = sb.tile([8, CH], F32, tag="nul")
                nc.sync.dma_start(nulltile[:], rb.ap())
                stash(nulltile[:], 8)
            if phase == "router":
                stash(scp[0:64, :], 64)

            if phase in ("rs", "topk", "wsel", "full", "timing"):
                scf = sb.tile([BS, CH], F32, tag="scf")
                if phase == "timing" or skip_cc:
                    # cost-model variant: skip the collective (~+12us on HW)
                    nc.vector.tensor_copy(scf[:], scp[0:BS, :])
                else:
                    rs_in = dram.tile([B, CH], F32)
                    rs_out = dram.tile([BS, CH], F32)
                    nc.sync.dma_start(rs_in[:], scp[:])
                    nc.gpsimd.collective_compute(
                        "ReduceScatter", OP.add,
                        replica_groups=[list(range(NCORES))],
                        ins=[rs_in.opt()], outs=[rs_out.opt()],
                    )
                    nc.sync.dma_start(scf[:], rs_out[:])
                nc.vector.tensor_tensor(scf[:], scf[:], rbt[:], OP.add)
            if phase == "rs":
                stash(scf[:], BS)

            if phase in ("topk", "wsel", "full", "timing"):
                # ---------------- phase T: exact top-128 ----------------
                sa = sb.tile([BS, CH], F32, tag="sa")
                nc.scalar.activation(sa[:], scf[:], AFT.Abs)
                lo = sb.tile([BS, 1], I32, tag="lo")
                nc.vector.memset(lo[:], 0)
                cand = sb.tile([BS, 1], I32, tag="cand")
                msk = sb.tile([BS, CH], F32, tag="msk")
                cnt = sb.tile([BS, 1], F32, tag="cnt")
                flag = sb.tile([BS, 1], F32, tag="flag")
                stpi = sb.tile([BS, 1], I32, tag="stpi")
                for b in range(30, -1, -1):
                    nc.vector.tensor_scalar(cand[:], lo[:], (1 << b), None,
                                            OP.add)
                    nc.vector.tensor_scalar(msk[:], sa[:],
                                            cand[:].bitcast(F32),
                                            None, OP.is_ge, OP.add,
                                            accum_out=cnt[:])
                    nc.vector.tensor_scalar(flag[:], cnt[:], float(COUT),
                                            float(1 << b), OP.is_ge, OP.mult)
                    nc.vector.tensor_copy(stpi[:], flag[:])
                    nc.vector.tensor_tensor(lo[:], lo[:], stpi[:], OP.add)
                # lo == bits of the 128th largest |score| per sample
                mgt = sb.tile([BS, CH], F32, tag="mgt")
                ngt = sb.tile([BS, 1], F32, tag="ngt")
                nc.vector.tensor_scalar(mgt[:], sa[:], lo[:].bitcast(F32),
                                        None, OP.is_gt, OP.add,
                                        accum_out=ngt[:])
                meq = sb.tile([BS, CH], F32, tag="meq")
                nc.vector.tensor_scalar(meq[:], sa[:], lo[:].bitcast(F32),
                                        None, OP.is_equal)
                need = sb.tile([BS, 1], F32, tag="need")
                nc.vector.tensor_scalar(need[:], ngt[:], -1.0, None, OP.mult)
                nc.vector.tensor_scalar(need[:], need[:], float(COUT), None,
                                        OP.add)
                zf = sb.tile([BS, CH], F32, tag="zf")
                nc.vector.memset(zf[:], 0.0)
                cume = sb.tile([BS, CH], F32, tag="cume")
                nc.vector.tensor_tensor_scan(cume[:], meq[:], zf[:], 0.0,
                                             OP.add, OP.add)
                keep = sb.tile([BS, CH], F32, tag="keep")
                nc.vector.tensor_scalar(keep[:], cume[:], need[:], None,
                                        OP.is_le)
                nc.vector.tensor_tensor(keep[:], keep[:], meq[:], OP.mult)
                nc.vector.tensor_tensor(msk[:], mgt[:], keep[:], OP.add)
                cum = sb.tile([BS, CH], F32, tag="cum")
                nc.vector.tensor_tensor_scan(cum[:], msk[:], zf[:], 0.0,
                                             OP.add, OP.add)
                pos = sb.tile([BS, CH], F32, tag="pos")
                nc.vector.tensor_tensor(pos[:], cum[:], msk[:], OP.mult)
                nc.vector.tensor_scalar(pos[:], pos[:], -1.0, None, OP.add)

                posT = sb.tile([128, 4, BS], F32, tag="posT")
                for c in range(4):
                    ptr = ps_tr.tile([128, BS], F32, tag="ptr")
                    nc.tensor.transpose(ptr[:], pos[:, c * 128:(c + 1) * 128],
                                        eyet[:])
                    nc.vector.tensor_copy(posT[:, c, :], ptr[:])
            if phase == "topk":
                stash(pos[:], BS)

            if phase in ("wsel", "full", "timing"):
                # ------------ phase S: selection matrices + weight gather ----
                S = sb.tile([128, 4, BS, 128], F32R, tag="S")
                for c in range(4):
                    for s in range(BS):
                        nc.vector.tensor_scalar(S[:, c, s, :], iott[:],
                                                posT[:, c, s:s + 1], None,
                                                OP.is_equal)
                # gathered weights wsel[m-chunk][s][j]; chunk 6 row 0 = bias
                wsel = sb.tile([128, 7, BS, 128], F32R, tag="wsel")
                for g in range(2):          # groups of 4 samples (N=512)
                    for m in range(7):
                        pw = ps_ws.tile([128, 4, 128], F32, tag="pw")
                        for c in range(4):
                            nc.tensor.matmul(
                                pw[:], wat[:, c, m * 128:(m + 1) * 128],
                                S[:, c, 4 * g:4 * g + 4, :],
                                start=(c == 0), stop=(c == 3))
                        nc.scalar.copy(wsel[:, m, 4 * g:4 * g + 4, :], pw[:])
                # bias row -> per-partition column via partition-scatter DMA
                bsel = sb.tile([128, BS], F32, tag="bsel")
                for s in range(BS):
                    nc.sync.dma_start(bsel[:, s:s + 1],
                                      wsel[0:1, 6, s, :].bitcast(F32))
            if phase == "wsel":
                stash(wsel[:, 0, 0, :].bitcast(F32), 128)

            if phase in ("full", "timing"):
                # ------------ phase C: conv on selected channels ------------
                for s in range(BS):
                    xx = sbxx.tile([128, HP, HP], F32R, tag="xx")
                    xxf = xx[:].bitcast(F32)
                    nc.gpsimd.memset(xxf[:, :, 0:1], 0.0)
                    nc.gpsimd.memset(xxf[:, :, 57:58], 0.0)
                    nc.gpsimd.memset(xxf[0:64, 0:1, 1:57], 0.0)
                    nc.gpsimd.memset(xxf[0:64, 57:58, 1:57], 0.0)
                    nc.gpsimd.memset(xxf[64:128, 56:58, 1:57], 0.0)
                    # stage x via fast sync DMA, cast f32->f32r on DVE
                    xst = sbxs.tile([128, H, W], F32, tag="xst")
                    nc.sync.dma_start(xst[0:64, :, :], xc.ap()[s])
                    nc.sync.dma_start(xst[64:128, :, :], xc.ap()[s])
                    # lower: x_pad rows; upper: x_pad shifted one row up
                    nc.vector.tensor_copy(xx[0:64, 1:57, 1:57], xst[0:64, :, :])
                    nc.vector.tensor_copy(xx[64:128, 0:56, 1:57],
                                          xst[64:128, :, :])
                    for tl in range(RT):
                        r0 = 1 + RPT * tl
                        pcv = ps_cv.tile([128, RPT, W], F32, tag="pcv")
                        for dx in range(3):
                            # dy0 (lower, rows r-1) + dy1 (upper slot r-1)
                            nc.tensor.matmul(
                                pcv[:], wsel[:, dx, s, :],
                                xx[:, r0 - 1:r0 + RPT - 1, dx:dx + W],
                                start=(dx == 0), stop=False)
                        for dx in range(3):
                            # dy2 = rows r+1
                            if tl % 2 == 0:
                                nc.tensor.matmul(
                                    pcv[:], wsel[0:64, 3 + dx, s, :],
                                    xx[0:64, r0 + 1:r0 + RPT + 1, dx:dx + W],
                                    start=False, stop=(dx == 2))
                            else:
                                nc.tensor.matmul(
                                    pcv[:], wsel[64:128, 3 + dx, s, :],
                                    xx[64:128, r0:r0 + RPT, dx:dx + W],
                                    start=False, stop=(dx == 2))
                        ot = sbot.tile([128, RPT, W], F32, tag="ot")
                        nc.scalar.activation(ot[:], pcv[:], AFT.Identity,
                                             bias=bsel[:, s:s + 1], scale=1.0)
                        nc.sync.dma_start(
                            out.ap()[s, :, RPT * tl:RPT * tl + RPT, :], ot[:])

    nc.compile()
    return nc


def _prep_inputs(x, conv_w, conv_b, router_w, router_b):
    x = np.asarray(x, dtype=np.float32)
    conv_w = np.asarray(conv_w, dtype=np.float32)
    conv_b = np.asarray(conv_b, dtype=np.float32)
    router_w = np.asarray(router_w, dtype=np.float32)
    router_b = np.asarray(router_b, dtype=np.float32)

    x_flat = x.reshape(B, IN_DIM)
    xK = x_flat.reshape(B, IN_DIM // 128, 128)          # [s, K, p]
    rwT = np.ascontiguousarray(
        router_w.reshape(CH, IN_DIM // 128, 128).transpose(1, 2, 0))  # [K,p,co]

    w4 = conv_w.reshape(CH, CIN, 3, 3)
    wam = np.zeros((CH, 896), np.float32)
    for t in range(3):
        wam[:, t * 128:t * 128 + 64] = w4[:, :, 0, t]        # dy0
        wam[:, t * 128 + 64:t * 128 + 128] = w4[:, :, 1, t]  # dy1
        wam[:, (3 + t) * 128:(3 + t) * 128 + 64] = w4[:, :, 2, t]
        wam[:, (3 + t) * 128 + 64:(3 + t) * 128 + 128] = w4[:, :, 2, t]
    wam[:, 768] = conv_b.reshape(CH)
    wa_dev = np.ascontiguousarray(wam.reshape(4, 128, 896))
    cb_dev = np.ascontiguousarray(conv_b.reshape(4, 128, 1))
    rb_dev = np.ascontiguousarray(
        np.broadcast_to(router_b[None, :], (BS, CH)))
    eye8 = np.eye(8, dtype=np.float32)
    iotaj = np.ascontiguousarray(
        np.broadcast_to(np.arange(128, dtype=np.float32)[None, :], (128, 128)))

    in_maps = []
    for r in range(NCORES):
        ks = slice(KC * r, KC * (r + 1))
        in_maps.append({
            "rw": np.ascontiguousarray(rwT[ks]),
            "xr": np.ascontiguousarray(xK[:, ks, :].transpose(2, 1, 0)),
            "xc": np.ascontiguousarray(x[BS * r:BS * (r + 1)]),
            "wa": wa_dev, "cb": cb_dev, "rb": rb_dev,
            "eye8": eye8, "iotaj": iotaj,
        })
    return in_maps


_NC_CACHE = None


def kernel(x, conv_w, conv_b, router_w, router_b):
    global _NC_CACHE
    if _NC_CACHE is None:
        _NC_CACHE = build_nc()
    nc = _NC_CACHE
    in_maps = _prep_inputs(x, conv_w, conv_b, router_w, router_b)
    res = run_bass_kernel_spmd(nc, in_maps, core_ids=list(range(NCORES)))
    return np.concatenate(
        [res.results[r]["out"] for r in range(NCORES)], axis=0)


# revision 43
# speedup vs baseline: 150.9305x; 1.0027x over previous
"""MoELayer Trainium2 kernel (8 NeuronCores, SPMD).

Strategy:
  - Router matmul row-sharded over in_dim: each core computes partial scores
    for ALL 64 samples over its 25088-wide slice (fp32, exact), then a
    ReduceScatter(add) hands each core the final scores of its own 8 samples.
  - Exact top-128 per sample via bit-bisection on |scores| (int32 view of
    fp32 is order-isomorphic for non-negative floats), with jax.top_k tie
    semantics (lowest index wins) via an equality-cumsum pass.
  - Per-sample one-hot selection matrix S [512, 128] built on DVE; conv
    weights gathered as w_sel = wa.T @ S with float32r matmuls (values are
    0/1 so S is exact; weights round to f32r ~13-bit mantissa).
  - 3x3 conv on the 128 selected channels only (4x compute saving) in
    float32r: "double image" SBUF layout xx = [x_pad ; x_pad shifted one row]
    so (dy=0, dy=1) pack into one K=128 matmul; dy=2 runs as K=64 matmuls
    alternating between the two partition halves (row-tiling overlap).
  - PSUM drained by ScalarE with fused per-channel bias add.

Batch is data-parallel: core r owns samples [8r, 8r+8).
"""
import numpy as np

import concourse.bacc as bacc
import concourse.bass as bass
import concourse.mybir as mybir
import concourse.tile as tile
from concourse.bass_utils import run_bass_kernel_spmd

F32 = mybir.dt.float32
F32R = mybir.dt.float32r
I32 = mybir.dt.int32
OP = mybir.AluOpType
AFT = mybir.ActivationFunctionType

B, CIN, H, W = 64, 64, 56, 56
COUT, NEXP = 128, 4
CH = NEXP * COUT            # 512
IN_DIM = CIN * H * W        # 200704
NCORES = 8
BS = B // NCORES            # 8 samples per core
KC = IN_DIM // NCORES // 128  # 196 k-chunks of 128 per core
HP = H + 2                  # 58 padded
RT = 7                      # row-tiles per sample (8 output rows each)
RPT = H // RT               # 8 rows per tile


def build_nc(phase="full", num_devices=NCORES, skip_cc=False):
    nc = bacc.Bacc("TRN2", target_bir_lowering=False, debug=False,
                   num_devices=num_devices)

    rw = nc.dram_tensor("rw", [KC, 128, CH], F32, kind="ExternalInput")
    xr = nc.dram_tensor("xr", [128, KC, B], F32, kind="ExternalInput")
    xc = nc.dram_tensor("xc", [BS, CIN, H, W], F32, kind="ExternalInput")
    wa = nc.dram_tensor("wa", [4, 128, 896], F32, kind="ExternalInput")
    cb = nc.dram_tensor("cb", [4, 128, 1], F32, kind="ExternalInput")
    rb = nc.dram_tensor("rb", [BS, CH], F32, kind="ExternalInput")
    eye8 = nc.dram_tensor("eye8", [8, 8], F32, kind="ExternalInput")
    iotaj = nc.dram_tensor("iotaj", [128, 128], F32, kind="ExternalInput")
    out = nc.dram_tensor("out", [BS, COUT, H, W], F32, kind="ExternalOutput")

    with tile.TileContext(nc) as tc:
        with (
            tc.tile_pool(name="sb", bufs=1) as sb,
            tc.tile_pool(name="sbrw", bufs=8) as sbrw,
            tc.tile_pool(name="sbxx", bufs=2) as sbxx,
            tc.tile_pool(name="sbxs", bufs=2) as sbxs,
            tc.tile_pool(name="sbot", bufs=3) as sbot,
            tc.tile_pool(name="dram", bufs=1, space="DRAM") as dram,
            tc.tile_pool(name="ps_sc", bufs=1, space="PSUM") as ps_sc,
            tc.tile_pool(name="ps_tr", bufs=1, space="PSUM") as ps_tr,
            tc.tile_pool(name="ps_ws", bufs=2, space="PSUM") as ps_ws,
            tc.tile_pool(name="ps_cv", bufs=3, space="PSUM") as ps_cv,
        ):
            # ---------------- constants / static loads ----------------
            eyet = sb.tile([8, 8], F32, tag="eye")
            nc.sync.dma_start(eyet[:], eye8.ap())
            iott = sb.tile([128, 128], F32, tag="iot")
            nc.sync.dma_start(iott[:], iotaj.ap())
            rbt = sb.tile([BS, CH], F32, tag="rb")
            nc.sync.dma_start(rbt[:], rb.ap())

            def stash(ap2d, rows):
                """debug drain of a [rows, F] 2D AP into `out`."""
                f = ap2d.free_size()
                cwid = max(1, f // 16)
                nc.sync.dma_start(
                    out.ap()[0, 0:rows, 0:f // cwid, 0:cwid],
                    ap2d.rearrange("p (a c) -> p a c", c=cwid))

            # ---------------- phase R: router partial scores ----------------
            if phase != "null":
                xrt = sb.tile([128, KC, B], F32, tag="xr")
                for kk in range(0, KC, 7):
                    nc.scalar.dma_start(xrt[:, kk:kk + 7, :],
                                        xr.ap()[:, kk:kk + 7, :])
                psc = ps_sc.tile([B, CH], F32, tag="psc")
                for k in range(KC):
                    rwk = sbrw.tile([128, CH], F32, tag="rwk")
                    nc.sync.dma_start(rwk[:], rw.ap()[k])
                    nc.tensor.matmul(psc[:], xrt[:, k, :], rwk[:],
                                     start=(k == 0), stop=(k == KC - 1))
                scp = sb.tile([B, CH], F32, tag="scp")
                nc.vector.tensor_copy(scp[:], psc[:])

            if phase == "null":
                nulltile = sb.tile([8, CH], F32, tag="nul")
                nc.sync.dma_start(nulltile[:], rb.ap())
                stash(nulltile[:], 8)
            if phase == "router":
                stash(scp[0:64, :], 64)

            if phase in ("rs", "topk", "wsel", "full", "timing"):
                scf = sb.tile([BS, CH], F32, tag="scf")
                if phase == "timing" or skip_cc:
                    # cost-model variant: skip the collective (~+12us on HW)
                    nc.vector.tensor_copy(scf[:], scp[0:BS, :])
                else:
                    rs_in = dram.tile([B, CH], F32)
                    rs_out = dram.tile([BS, CH], F32)
                    nc.sync.dma_start(rs_in[:], scp[:])
                    nc.gpsimd.collective_compute(
                        "ReduceScatter", OP.add,
                        replica_groups=[list(range(NCORES))],
                        ins=[rs_in.opt()], outs=[rs_out.opt()],
                    )
                    nc.sync.dma_start(scf[:], rs_out[:])
                nc.vector.tensor_tensor(scf[:], scf[:], rbt[:], OP.add)
            if phase == "rs":
                stash(scf[:], BS)

            if phase in ("topk", "wsel", "full", "timing"):
                # ---------------- phase T: exact top-128 ----------------
                sa = sb.tile([BS, CH], F32, tag="sa")
                nc.scalar.activation(sa[:], scf[:], AFT.Abs)
                lo = sb.tile([BS, 1], I32, tag="lo")
                nc.vector.memset(lo[:], 0)
                cand = sb.tile([BS, 1], I32, tag="cand")
                msk = sb.tile([BS, CH], F32, tag="msk")
                cnt = sb.tile([BS, 1], F32, tag="cnt")
                flag = sb.tile([BS, 1], F32, tag="flag")
                stpi = sb.tile([BS, 1], I32, tag="stpi")
                for b in range(30, -1, -1):
                    nc.vector.tensor_scalar(cand[:], lo[:], (1 << b), None,
                                            OP.add)
                    nc.vector.tensor_scalar(msk[:], sa[:],
                                            cand[:].bitcast(F32),
                                            None, OP.is_ge, OP.add,
                                            accum_out=cnt[:])
                    nc.vector.tensor_scalar(flag[:], cnt[:], float(COUT),
                                            float(1 << b), OP.is_ge, OP.mult)
                    nc.vector.tensor_copy(stpi[:], flag[:])
                    nc.vector.tensor_tensor(lo[:], lo[:], stpi[:], OP.add)
                # lo == bits of the 128th largest |score| per sample
                mgt = sb.tile([BS, CH], F32, tag="mgt")
                ngt = sb.tile([BS, 1], F32, tag="ngt")
                nc.vector.tensor_scalar(mgt[:], sa[:], lo[:].bitcast(F32),
                                        None, OP.is_gt, OP.add,
                                        accum_out=ngt[:])
                meq = sb.tile([BS, CH], F32, tag="meq")
                nc.vector.tensor_scalar(meq[:], sa[:], lo[:].bitcast(F32),
                                        None, OP.is_equal)
                need = sb.tile([BS, 1], F32, tag="need")
                nc.vector.tensor_scalar(need[:], ngt[:], -1.0, None, OP.mult)
                nc.vector.tensor_scalar(need[:], need[:], float(COUT), None,
                                        OP.add)
                zf = sb.tile([BS, CH], F32, tag="zf")
                nc.vector.memset(zf[:], 0.0)
                cume = sb.tile([BS, CH], F32, tag="cume")
                nc.vector.tensor_tensor_scan(cume[:], meq[:], zf[:], 0.0,
                                             OP.add, OP.add)
                keep = sb.tile([BS, CH], F32, tag="keep")
                nc.vector.tensor_scalar(keep[:], cume[:], need[:], None,
                                        OP.is_le)
                nc.vector.tensor_tensor(keep[:], keep[:], meq[:], OP.mult)
                nc.vector.tensor_tensor(msk[:], mgt[:], keep[:], OP.add)
                cum = sb.tile([BS, CH], F32, tag="cum")
                nc.vector.tensor_tensor_scan(cum[:], msk[:], zf[:], 0.0,
                                             OP.add, OP.add)
                pos = sb.tile([BS, CH], F32, tag="pos")
                nc.vector.tensor_tensor(pos[:], cum[:], msk[:], OP.mult)
                nc.vector.tensor_scalar(pos[:], pos[:], -1.0, None, OP.add)

                posT = sb.tile([128, 4, BS], F32, tag="posT")
                for c in range(4):
                    ptr = ps_tr.tile([128, BS], F32, tag="ptr")
                    nc.tensor.transpose(ptr[:], pos[:, c * 128:(c + 1) * 128],
                                        eyet[:])
                    nc.vector.tensor_copy(posT[:, c, :], ptr[:])
            if phase == "topk":
                stash(pos[:], BS)

            if phase in ("wsel", "full", "timing"):
                # ------------ phase S: selection matrices + weight gather ----
                wat = sb.tile([128, 4, 896], F32R, tag="wa")
                for c in range(4):
                    nc.gpsimd.dma_start(wat[:, c, :], wa.ap()[c])
                S = sb.tile([128, 4, BS, 128], F32R, tag="S")
                for c in range(4):
                    for s in range(BS):
                        nc.vector.tensor_scalar(S[:, c, s, :], iott[:],
                                                posT[:, c, s:s + 1], None,
                                                OP.is_equal)
                # gathered weights wsel[m-chunk][s][j]; chunk 6 row 0 = bias
                wsel = sb.tile([128, 7, BS, 128], F32R, tag="wsel")
                for g in range(2):          # groups of 4 samples (N=512)
                    for m in range(7):
                        pw = ps_ws.tile([128, 4, 128], F32, tag="pw")
                        for c in range(4):
                            nc.tensor.matmul(
                                pw[:], wat[:, c, m * 128:(m + 1) * 128],
                                S[:, c, 4 * g:4 * g + 4, :],
                                start=(c == 0), stop=(c == 3))
                        nc.scalar.copy(wsel[:, m, 4 * g:4 * g + 4, :], pw[:])
                # bias row -> per-partition column via partition-scatter DMA
                bsel = sb.tile([128, BS], F32, tag="bsel")
                for s in range(BS):
                    nc.sync.dma_start(bsel[:, s:s + 1],
                                      wsel[0:1, 6, s, :].bitcast(F32))
            if phase == "wsel":
                stash(wsel[:, 0, 0, :].bitcast(F32), 128)

            if phase in ("full", "timing"):
                # ------------ phase C: conv on selected channels ------------
                for s in range(BS):
                    xx = sbxx.tile([128, HP, HP], F32R, tag="xx")
                    xxf = xx[:].bitcast(F32)
                    nc.gpsimd.memset(xxf[:, :, 0:1], 0.0)
                    nc.gpsimd.memset(xxf[:, :, 57:58], 0.0)
                    nc.gpsimd.memset(xxf[0:64, 0:1, 1:57], 0.0)
                    nc.gpsimd.memset(xxf[0:64, 57:58, 1:57], 0.0)
                    nc.gpsimd.memset(xxf[64:128, 56:58, 1:57], 0.0)
                    # stage x via fast sync DMA, cast f32->f32r on DVE
                    xst = sbxs.tile([128, H, W], F32, tag="xst")
                    nc.sync.dma_start(xst[0:64, :, :], xc.ap()[s])
                    nc.sync.dma_start(xst[64:128, :, :], xc.ap()[s])
                    # lower: x_pad rows; upper: x_pad shifted one row up
                    nc.vector.tensor_copy(xx[0:64, 1:57, 1:57], xst[0:64, :, :])
                    nc.vector.tensor_copy(xx[64:128, 0:56, 1:57],
                                          xst[64:128, :, :])
                    for tl in range(RT):
                        r0 = 1 + RPT * tl
                        pcv = ps_cv.tile([128, RPT, W], F32, tag="pcv")
                        for dx in range(3):
                            # dy0 (lower, rows r-1) + dy1 (upper slot r-1)
                            nc.tensor.matmul(
                                pcv[:], wsel[:, dx, s, :],
                                xx[:, r0 - 1:r0 + RPT - 1, dx:dx + W],
                                start=(dx == 0), stop=False)
                        for dx in range(3):
                            # dy2 = rows r+1
                            if tl % 2 == 0:
                                nc.tensor.matmul(
                                    pcv[:], wsel[0:64, 3 + dx, s, :],
                                    xx[0:64, r0 + 1:r0 + RPT + 1, dx:dx + W],
                                    start=False, stop=(dx == 2))
                            else:
                                nc.tensor.matmul(
                                    pcv[:], wsel[64:128, 3 + dx, s, :],
                                    xx[64:128, r0:r0 + RPT, dx:dx + W],
                                    start=False, stop=(dx == 2))
                        ot = sbot.tile([128, RPT, W], F32, tag="ot")
                        nc.scalar.activation(ot[:], pcv[:], AFT.Identity,
                                             bias=bsel[:, s:s + 1], scale=1.0)
                        nc.sync.dma_start(
                            out.ap()[s, :, RPT * tl:RPT * tl + RPT, :], ot[:])

    nc.compile()
    return nc


def _prep_inputs(x, conv_w, conv_b, router_w, router_b):
    x = np.asarray(x, dtype=np.float32)
    conv_w = np.asarray(conv_w, dtype=np.float32)
    conv_b = np.asarray(conv_b, dtype=np.float32)
    router_w = np.asarray(router_w, dtype=np.float32)
    router_b = np.asarray(router_b, dtype=np.float32)

    x_flat = x.reshape(B, IN_DIM)
    xK = x_flat.reshape(B, IN_DIM // 128, 128)          # [s, K, p]
    rwT = np.ascontiguousarray(
        router_w.reshape(CH, IN_DIM // 128, 128).transpose(1, 2, 0))  # [K,p,co]

    w4 = conv_w.reshape(CH, CIN, 3, 3)
    wam = np.zeros((CH, 896), np.float32)
    for t in range(3):
        wam[:, t * 128:t * 128 + 64] = w4[:, :, 0, t]        # dy0
        wam[:, t * 128 + 64:t * 128 + 128] = w4[:, :, 1, t]  # dy1
        wam[:, (3 + t) * 128:(3 + t) * 128 + 64] = w4[:, :, 2, t]
        wam[:, (3 + t) * 128 + 64:(3 + t) * 128 + 128] = w4[:, :, 2, t]
    wam[:, 768] = conv_b.reshape(CH)
    wa_dev = np.ascontiguousarray(wam.reshape(4, 128, 896))
    cb_dev = np.ascontiguousarray(conv_b.reshape(4, 128, 1))
    rb_dev = np.ascontiguousarray(
        np.broadcast_to(router_b[None, :], (BS, CH)))
    eye8 = np.eye(8, dtype=np.float32)
    iotaj = np.ascontiguousarray(
        np.broadcast_to(np.arange(128, dtype=np.float32)[None, :], (128, 128)))

    in_maps = []
    for r in range(NCORES):
        ks = slice(KC * r, KC * (r + 1))
        in_maps.append({
            "rw": np.ascontiguousarray(rwT[ks]),
            "xr": np.ascontiguousarray(xK[:, ks, :].transpose(2, 1, 0)),
            "xc": np.ascontiguousarray(x[BS * r:BS * (r + 1)]),
            "wa": wa_dev, "cb": cb_dev, "rb": rb_dev,
            "eye8": eye8, "iotaj": iotaj,
        })
    return in_maps


_NC_CACHE = None


def kernel(x, conv_w, conv_b, router_w, router_b):
    global _NC_CACHE
    if _NC_CACHE is None:
        _NC_CACHE = build_nc()
    nc = _NC_CACHE
    in_maps = _prep_inputs(x, conv_w, conv_b, router_w, router_b)
    res = run_bass_kernel_spmd(nc, in_maps, core_ids=list(range(NCORES)))
    return np.concatenate(
        [res.results[r]["out"] for r in range(NCORES)], axis=0)
